# revision 57
# baseline (speedup 1.0000x reference)
"""GCN edge classifier on 8 TRN2 NeuronCores (Bass/Tile).

Math (PyG GCNConv x2 + edge MLP), with deg including self-loop:
    dinv = 1/sqrt(deg)
    g1 = dinv * (x @ W1);         agg1[c] = sum_{e: col=c} g1[row_e]
    h1 = relu(dinv * (agg1 + g1) + b1)
    g2 = dinv * (h1 @ W2);        agg2 likewise
    h2 = relu(dinv * (agg2 + g2) + b2)
    score_e = relu(A[row_e] + B[col_e]) @ w4 + b4,
        A = h2 @ W3[:64] + b3,  B = h2 @ W3[64:]

Distribution: edges bucketed by target (col) range -> owning core; each core
owns 12500 nodes.  Conv aggregation: edges sorted by (window, src-chunk,
target-tile); dma_gather (256B bf16 rows, int16 chunk-local indices) pulls
messages into SBUF blocks of 128 edges; a host-built one-hot selection block
[128 edges x 128 targets] is the stationary matmul operand, the message block
the moving operand, so TensorE accumulates agg[128 targets, 64] in PSUM per
node tile (scatter side runs at ~1 cyc/edge instead of ~8 ns/edge SWDGE).
g2/A slices are AllGathered between layers.  Edge MLP unchanged: dma_gather
A/B rows per edge.
"""
from dataclasses import dataclass, field

import numpy as np
import ml_dtypes

NCORES = 8
HID = 64
IN_DIM = 3
TPW = 16          # node tiles per PSUM window
NQ = 4            # SWDGE queues: gather desc-gen round-robins over Q7 core pairs

# MLP caps retained from the fixed-seed inputs (host prep asserts fit).
CAP_MLP_J = [135168, 135168, 135168, 10240]


@dataclass
class Cfg:
    n_nodes: int = 100000
    npc: int = 12500          # nodes per core
    chunk: int = 32768        # gather chunk rows (int16 index range)
    cap_mlp_j: list = field(default_factory=lambda: CAP_MLP_J)
    mlp_blk: int = 16384      # MLP block edges
    p1_grp: int = 8           # table-build tiles per PSUM batch

    @property
    def nchunk(self):
        return -(-self.n_nodes // self.chunk)

    @property
    def npc_pad(self):        # accum ids per core rounded to 128
        return -(-self.npc // 128) * 128

    @property
    def own_tiles(self):
        return self.npc_pad // 128

    @property
    def nwin(self):
        return -(-self.own_tiles // TPW)

    @property
    def e_mlp(self):
        return int(sum(self.cap_mlp_j))

    @property
    def nt(self):             # table tiles (128 rows each), grouped by p1_grp
        raw = -(-self.n_nodes // 128)
        g = self.p1_grp
        return -(-raw // g) * g

    @property
    def tbl_rows(self):
        return self.nt * 128

    def chunk_bounds(self, j):
        lo = j * self.chunk
        return lo, min((j + 1) * self.chunk, self.n_nodes)


REAL = Cfg()

_COMPILED = {}


def build_program(cfg: Cfg, caps, mlp_caps, k_pos):
    """caps[w][j][gl] = padded edge count (multiple of 128) for window w,
    src chunk j, in-window tile gl.  mlp_caps[j][gt] likewise for the edge
    MLP stream keyed by (src chunk, target tile).  k_pos = number of hid
    channels with w4 >= 0 after the host's sign-permutation fold."""
    import concourse.bacc as bacc
    import concourse.mybir as mybir
    import concourse.tile as tile
    from concourse.library_config import mlp as mlp_lib

    F32 = mybir.dt.float32
    BF16 = mybir.dt.float16  # 2-byte msg path: fp16 (8x finer mantissa than bf16)
    F8 = mybir.dt.float8e4   # one-hot selection streams: fp8 is exact for 0/1
    I16 = mybir.dt.int16
    AOp = mybir.AluOpType

    NW = cfg.nwin
    wj_slots = [[sum(caps[w][j]) for j in range(cfg.nchunk)] for w in range(NW)]
    e_conv = int(sum(sum(r) for r in wj_slots))
    maxwj = max(max(r) for r in wj_slots)
    # total matmul blocks per (w, gl) across chunks, for start/stop flags
    blk_tot = [[sum(caps[w][j][gl] // 128 for j in range(cfg.nchunk))
                for gl in range(TPW)] for w in range(NW)]
    e_mlp = int(sum(sum(r) for r in mlp_caps))
    capmax2 = max(max(r) for r in mlp_caps)
    nblk_max = capmax2 // 128

    nc = bacc.Bacc("TRN2", target_bir_lowering=False, debug=False,
                   num_devices=NCORES, num_swdge_queues=NQ)

    # ---- I/O ----
    xTp = nc.dram_tensor("xTp", [IN_DIM, cfg.tbl_rows], BF16, kind="ExternalInput")
    xTo = nc.dram_tensor("xTo", [IN_DIM, cfg.npc_pad], BF16, kind="ExternalInput")
    degp = nc.dram_tensor("degp", [128, cfg.nt], F32, kind="ExternalInput")
    dego = nc.dram_tensor("dego", [128, cfg.own_tiles], F32, kind="ExternalInput")
    W1 = nc.dram_tensor("W1", [IN_DIM, HID], BF16, kind="ExternalInput")
    W2 = nc.dram_tensor("W2", [HID, HID], F32, kind="ExternalInput")
    W3t = nc.dram_tensor("W3t", [HID, HID], F32, kind="ExternalInput")
    W3b = nc.dram_tensor("W3b", [HID, HID], F32, kind="ExternalInput")
    b1r = nc.dram_tensor("b1r", [128, HID], F32, kind="ExternalInput")
    b2r = nc.dram_tensor("b2r", [128, HID], F32, kind="ExternalInput")
    b3r = nc.dram_tensor("b3r", [128, HID], F32, kind="ExternalInput")
    w4r = nc.dram_tensor("w4r", [128, HID], F32, kind="ExternalInput")
    b4r = nc.dram_tensor("b4r", [128, 1], F32, kind="ExternalInput")
    idm = nc.dram_tensor("idm", [128, 128], F32, kind="ExternalInput")
    cg = nc.dram_tensor("cg", [128, e_conv // 16], I16, kind="ExternalInput")
    s1d = nc.dram_tensor("s1d", [128, e_conv], F8, kind="ExternalInput")
    cg2 = nc.dram_tensor("cg2", [128, e_conv // 16], I16, kind="ExternalInput")
    s1d2 = nc.dram_tensor("s1d2", [128, e_conv], F8, kind="ExternalInput")
    ma = nc.dram_tensor("ma", [128, e_mlp // 16], I16, kind="ExternalInput")
    s2d = nc.dram_tensor("s2d", [128, e_mlp], F8, kind="ExternalInput")
    scores = nc.dram_tensor("scores", [128, e_mlp // 128], F32,
                            kind="ExternalOutput")

    # ---- internal DRAM ----
    csz = [cfg.chunk_bounds(j)[1] - cfg.chunk_bounds(j)[0]
           for j in range(cfg.nchunk)]
    ctil = [-(-cfg.tbl_rows // 128) - sum(-(-s // 128) for s in csz[:3])
            if j == cfg.nchunk - 1 else csz[j] // 128
            for j in range(cfg.nchunk)]
    g1c = [nc.dram_tensor(f"g1c{j}", [ctil[j] * 128, 128], BF16)
           for j in range(cfg.nchunk)]
    g2s = nc.dram_tensor("g2s", [cfg.npc, 128], BF16)
    g2f = nc.dram_tensor("g2f", [cfg.n_nodes, 128], BF16, addr_space="Shared")
    As = nc.dram_tensor("As", [cfg.npc, HID], F32)
    Af = nc.dram_tensor("Af", [cfg.n_nodes, HID], F32, addr_space="Shared")
    Bl = nc.dram_tensor("Bl", [cfg.npc_pad, HID], BF16)

    CALL = 1024  # SWDGE descriptor-ring safe per-call index limit

    SUB = 8192  # slots per staged sub-batch

    qrr = [0]  # round-robin SWDGE queue cursor

    def next_q():
        q = qrr[0]
        qrr[0] = (q + 1) % NQ
        return q

    def conv_pass(tc, ip, mp, sp, pp, gsrc_slices, agg):
        # one PSUM accumulation group open at a time: groups are the
        # gl-consecutive blocks within a (w, j) slab; cross-chunk partials
        # are summed into agg (pre-zeroed) on the vector engine.
        # Sub-batches are software-pipelined: gix index loads issue two
        # sub-batches ahead on the scalar HWDGE ring so gathers never wait.
        nc_ = tc.nc
        slabs = []
        off = 0
        for j in range(cfg.nchunk):
            for w in range(NW):
                nwj = wj_slots[w][j]
                if nwj == 0:
                    continue
                slabs.append((w, j, nwj))
                off += nwj
        subs = []
        soff = 0
        for si, (w, j, nwj) in enumerate(slabs):
            for s0 in range(0, nwj, SUB):
                subs.append((si, s0, min(SUB, nwj - s0), soff + s0))
            soff += nwj
        gixs = {}

        def load_gix(n):
            if n >= len(subs):
                return
            _, _, ns, so = subs[n]
            gix = ip.tile([128, SUB // 16], I16, tag="gix")
            nc_.scalar.dma_start(gix[:, :ns // 16],
                                 cg[:, so // 16: so // 16 + ns // 16])
            gixs[n] = gix

        load_gix(0)
        load_gix(1)
        state = {}
        for n, (si, s0, ns, so) in enumerate(subs):
            load_gix(n + 2)
            w, j, nwj = slabs[si]
            if si not in state:
                nblk = [caps[w][j][gl] // 128 for gl in range(TPW)]
                pw_new = pp.tile([128, TPW, HID], F32, tag="pw")
                state[si] = (
                    pw_new, nblk,
                    [gl for gl in range(TPW) for _ in range(nblk[gl])],
                    [0] * TPW)
            pw, nblk, gl_of_blk, seen = state[si]
            gix = gixs.pop(n)
            mt = mp.tile([128, SUB // 128, 128], BF16, tag="msg")
            for q0 in range(0, ns, CALL):
                nn = min(CALL, ns - q0)
                nc_.gpsimd.dma_gather(
                    mt[:, q0 // 128: (q0 + nn) // 128, :],
                    gsrc_slices[j], gix[:, q0 // 16: (q0 + nn) // 16],
                    nn, nn, 128, queue_num=next_q())
            s1t = sp.tile([128, SUB // 128, 128], F8, tag="s1")
            nc_.sync.dma_start(
                s1t[:, :ns // 128, :], s1d[:, so: so + ns])
            for b in range(ns // 128):
                gl = gl_of_blk[(s0 // 128) + b]
                nc_.tensor.matmul(
                    pw[:, gl, :], s1t[:, b, :], mt[:, b, :HID],
                    start=(seen[gl] == 0),
                    stop=(seen[gl] == nblk[gl] - 1))
                seen[gl] += 1
            if s0 + SUB >= nwj:   # slab epilogue: drain PSUM via scalar,
                # one batched vector add (PSUM reads on vector are slow and
                # a busy vector throttles Q7 desc-gen via the shared port)
                valid = min(TPW, cfg.own_tiles - w * TPW)
                pwc = mp.tile([128, TPW, HID], F32, tag="pwc")
                nc_.scalar.copy(pwc[:, :valid], pw[:, :valid, :])
                nc_.vector.tensor_tensor(
                    agg[:, w * TPW: w * TPW + valid, :],
                    agg[:, w * TPW: w * TPW + valid, :],
                    pwc[:, :valid], op=AOp.add)
                del state[si]

    with tile.TileContext(nc) as tc:
        with (
            tc.tile_pool(name="const", bufs=1) as cp,
            tc.tile_pool(name="acc", bufs=1) as ap_,
            tc.tile_pool(name="dinv", bufs=1) as dp,
        ):
            nc.gpsimd.load_library(mlp_lib)

            W1s = cp.tile([IN_DIM, HID], BF16, tag="W1")
            W2s = cp.tile([HID, HID], F32, tag="W2")
            W3ts = cp.tile([HID, HID], F32, tag="W3t")
            W3bs = cp.tile([HID, HID], F32, tag="W3b")
            b1s = cp.tile([128, HID], F32, tag="b1")
            b2s = cp.tile([128, HID], F32, tag="b2")
            b3s = cp.tile([128, HID], F32, tag="b3")
            w4s = cp.tile([128, HID], F32, tag="w4")
            b4s = cp.tile([128, 1], F32, tag="b4")
            idms = cp.tile([128, 128], F32, tag="idm")
            for t_, d_ in ((W1s, W1), (W2s, W2), (W3ts, W3t), (W3bs, W3b),
                           (b1s, b1r), (b2s, b2r), (b3s, b3r), (w4s, w4r),
                           (b4s, b4r), (idms, idm)):
                nc.sync.dma_start(t_[:], d_[:])

            # dinv tables
            dvp = dp.tile([128, cfg.nt], F32, tag="dvp")
            dvo = dp.tile([128, cfg.own_tiles], F32, tag="dvo")
            for dst, src, n in ((dvp, degp, cfg.nt), (dvo, dego, cfg.own_tiles)):
                tmp = dp.tile([128, n], F32, tag="dtmp")
                nc.sync.dma_start(tmp[:], src[:])
                nc.vector.reciprocal(tmp[:], tmp[:])
                nc.scalar.sqrt(dst[:], tmp[:])

            agg = ap_.tile([128, cfg.own_tiles, HID], F32, tag="agg")
            nc.vector.memset(agg[:], 0.0)

            # ---- P1: build g1 table (replicated, rank-permuted layout) ----
            g1c_t = [t_[:].rearrange("(t p) e -> p t e", p=128)
                     for t_ in g1c]
            PG = cfg.p1_grp
            with (
                tc.tile_pool(name="p1", bufs=3) as p1p,
                tc.tile_pool(name="p1ps", bufs=4, space="PSUM") as p1ps,
            ):
                for tg in range(0, cfg.nt, PG):
                    xt = p1p.tile([IN_DIM, PG * 128], BF16, tag="xt")
                    nc.sync.dma_start(
                        xt[:], xTp[:, tg * 128: (tg + PG) * 128])
                    ps = p1ps.tile([128, PG, HID], F32, tag="p1b")
                    for i in range(PG):
                        nc.tensor.matmul(
                            ps[:, i, :], xt[:, i * 128: (i + 1) * 128], W1s[:])
                    # pad cols 64..127 stay garbage: every consumer reads
                    # only [:, :HID] of gathered rows
                    gt = p1p.tile([128, PG, 128], BF16, tag="g1t")
                    dv_b = dvp[:, tg: tg + PG, None].broadcast_to(
                        [128, PG, HID])
                    nc.vector.tensor_tensor(gt[:, :, :HID], ps[:], dv_b,
                                            op=AOp.mult)
                    jc = (tg * 128) // cfg.chunk
                    tl = tg - jc * (cfg.chunk // 128)
                    nc.sync.dma_start(g1c_t[jc][:, tl: tl + PG, :], gt[:])

            # ---- P2: conv1 aggregation ----
            g1_slices = [g1c[j][:csz[j], :] for j in range(cfg.nchunk)]
            with (
                tc.tile_pool(name="c1i", bufs=4) as ip,
                tc.tile_pool(name="c1m", bufs=4) as mp,
                tc.tile_pool(name="c1s", bufs=4) as sp,
                tc.tile_pool(name="c1p", bufs=2, space="PSUM") as pp,
            ):
                conv_pass(tc, ip, mp, sp, pp, g1_slices, agg)

            # ---- P3: pointwise h1, build g2 slice (batched 8 tiles) ----
            PB = 8
            with (
                tc.tile_pool(name="p3", bufs=3) as p3p,
                tc.tile_pool(name="idmp", bufs=1) as idp,
                tc.tile_pool(name="p3ps", bufs=2, space="PSUM") as p3ps,
            ):
                xto = idp.tile([IN_DIM, cfg.npc_pad], BF16, tag="xto")
                nc.sync.dma_start(xto[:], xTo[:])
                for t0 in range(0, cfg.own_tiles, PB):
                    nb = min(PB, cfg.own_tiles - t0)
                    hw1 = p3ps.tile([128, PB, HID], F32, tag="hw1")
                    for i in range(nb):
                        nc.tensor.matmul(
                            hw1[:, i, :],
                            xto[:, (t0 + i) * 128:(t0 + i + 1) * 128], W1s[:])
                    dv_b = dvo[:, t0:t0 + nb, None].broadcast_to(
                        [128, nb, HID])
                    u = p3p.tile([128, PB, HID], F32, tag="u")
                    nc.vector.tensor_tensor(u[:, :nb], hw1[:, :nb], dv_b,
                                            op=AOp.mult)
                    nc.vector.tensor_tensor(u[:, :nb], u[:, :nb],
                                            agg[:, t0:t0 + nb, :], op=AOp.add)
                    h1 = p3p.tile([128, PB, HID], F32, tag="h1")
                    nc.vector.tensor_tensor(h1[:, :nb], u[:, :nb], dv_b,
                                            op=AOp.mult)
                    b1_b = b1s[:, None, :].broadcast_to([128, nb, HID])
                    nc.vector.tensor_tensor(h1[:, :nb], h1[:, :nb], b1_b,
                                            op=AOp.add)
                    nc.vector.tensor_scalar_max(h1[:, :nb], h1[:, :nb], 0.0)
                    trp = p3ps.tile([HID, PB, 128], F32, tag="trp")
                    for i in range(nb):
                        nc.tensor.transpose(trp[:, i, :], h1[:, i, :], idms[:])
                    h1T = p3p.tile([HID, PB, 128], F32, tag="h1T")
                    nc.scalar.copy(h1T[:, :nb], trp[:, :nb])
                    hw2 = p3ps.tile([128, PB, HID], F32, tag="hw2")
                    for i in range(nb):
                        nc.tensor.matmul(hw2[:, i, :], h1T[:, i, :], W2s[:])
                    g2t = p3p.tile([128, PB, 128], BF16, tag="g2t")
                    nc.vector.tensor_tensor(g2t[:, :nb, :HID], hw2[:, :nb],
                                            dv_b, op=AOp.mult)
                    rows = min(128 * nb, cfg.npc - t0 * 128)
                    fr = rows // 128
                    if fr:
                        nc.sync.dma_start(
                            g2s[t0 * 128: (t0 + fr) * 128, :].rearrange(
                                "(i p) e -> p i e", p=128),
                            g2t[:, :fr, :])
                    rem = rows - fr * 128
                    if rem:
                        nc.sync.dma_start(
                            g2s[(t0 + fr) * 128: t0 * 128 + rows, :],
                            g2t[:rem, fr, :])

            nc.gpsimd.collective_compute(
                "AllGather", mybir.AluOpType.bypass,
                replica_groups=[list(range(NCORES))],
                ins=[g2s[:]], outs=[g2f[:]])

            # ---- P4+P5 fused: conv2 aggregation (w-major stream, window
            # accumulates in PSUM across chunks) + pointwise h2/A/B emitted
            # per finished window, overlapping the next window's gathers ----
            g2_slices = [g2f[lo:hi, :] for lo, hi in
                         (cfg.chunk_bounds(j) for j in range(cfg.nchunk))]
            with (
                tc.tile_pool(name="c2i", bufs=4) as ip,
                tc.tile_pool(name="c2m", bufs=4) as mp,
                tc.tile_pool(name="c2s", bufs=4) as sp,
                tc.tile_pool(name="c2p", bufs=2, space="PSUM") as pp,
                tc.tile_pool(name="p5", bufs=3) as p5p,
                tc.tile_pool(name="p5ps", bufs=1, space="PSUM") as p5ps,
            ):
                slabs = [(w, j, wj_slots[w][j]) for w in range(NW)
                         for j in range(cfg.nchunk) if wj_slots[w][j]]
                subs = []
                soff = 0
                for si, (w, j, nwj) in enumerate(slabs):
                    for s0 in range(0, nwj, SUB):
                        subs.append((si, s0, min(SUB, nwj - s0), soff + s0))
                    soff += nwj
                gixs = {}

                def load_gix2(n):
                    if n >= len(subs):
                        return
                    _, _, ns, so = subs[n]
                    gix = ip.tile([128, SUB // 16], I16, tag="gix2")
                    nc.scalar.dma_start(
                        gix[:, :ns // 16],
                        cg2[:, so // 16: so // 16 + ns // 16])
                    gixs[n] = gix

                def p5_window(w, pw):
                    t_lo = w * TPW
                    t_hi = min(t_lo + TPW, cfg.own_tiles)
                    for t0 in range(t_lo, t_hi, PB):
                        nb = min(PB, t_hi - t0)
                        rows = min(128 * nb, cfg.npc - t0 * 128)
                        fr = rows // 128
                        rem = rows - fr * 128
                        g2o = p5p.tile([128, PB, 128], BF16, tag="g2o")
                        if rem:
                            nc.vector.memset(g2o[:], 0.0)
                        if fr:
                            nc.sync.dma_start(
                                g2o[:, :fr, :],
                                g2s[t0 * 128: (t0 + fr) * 128, :].rearrange(
                                    "(i p) e -> p i e", p=128))
                        if rem:
                            nc.sync.dma_start(
                                g2o[:rem, fr, :],
                                g2s[(t0 + fr) * 128: t0 * 128 + rows, :])
                        g2c = p5p.tile([128, PB, HID], F32, tag="g2c")
                        nc.scalar.copy(g2c[:, :nb], g2o[:, :nb, :HID])
                        u = p5p.tile([128, PB, HID], F32, tag="u5")
                        nc.vector.tensor_tensor(
                            u[:, :nb], g2c[:, :nb],
                            agg[:, t0:t0 + nb, :], op=AOp.add)
                        dv_b = dvo[:, t0:t0 + nb, None].broadcast_to(
                            [128, nb, HID])
                        h2 = p5p.tile([128, PB, HID], F32, tag="h2")
                        nc.vector.tensor_tensor(h2[:, :nb], u[:, :nb], dv_b,
                                                op=AOp.mult)
                        b2_b = b2s[:, None, :].broadcast_to([128, nb, HID])
                        nc.vector.tensor_tensor(h2[:, :nb], h2[:, :nb], b2_b,
                                                op=AOp.add)
                        nc.vector.tensor_scalar_max(h2[:, :nb], h2[:, :nb],
                                                    0.0)
                        trp = p5ps.tile([HID, PB, 128], F32, tag="trp5")
                        for i in range(nb):
                            nc.tensor.transpose(trp[:, i, :], h2[:, i, :],
                                                idms[:])
                        h2T = p5p.tile([HID, PB, 128], F32, tag="h2T")
                        nc.scalar.copy(h2T[:, :nb], trp[:, :nb])
                        psA = p5ps.tile([128, PB, HID], F32, tag="psA")
                        for i in range(nb):
                            nc.tensor.matmul(psA[:, i, :], h2T[:, i, :],
                                             W3ts[:])
                        At = p5p.tile([128, PB, HID], F32, tag="At")
                        b3_b = b3s[:, None, :].broadcast_to([128, nb, HID])
                        nc.vector.tensor_tensor(At[:, :nb], psA[:, :nb], b3_b,
                                                op=AOp.add)
                        if fr:
                            nc.sync.dma_start(
                                As[t0 * 128: (t0 + fr) * 128, :].rearrange(
                                    "(i p) h -> p i h", p=128),
                                At[:, :fr, :])
                        if rem:
                            nc.sync.dma_start(
                                As[(t0 + fr) * 128: t0 * 128 + rows, :],
                                At[:rem, fr, :])
                        psB = p5ps.tile([128, PB, HID], F32, tag="psB")
                        for i in range(nb):
                            nc.tensor.matmul(psB[:, i, :], h2T[:, i, :],
                                             W3bs[:])
                        Bt = p5p.tile([128, PB, HID], BF16, tag="Bt")
                        nc.vector.tensor_copy(Bt[:, :nb], psB[:, :nb])
                        nc.sync.dma_start(
                            Bl[t0 * 128: (t0 + nb) * 128, :].rearrange(
                                "(i p) h -> p i h", p=128),
                            Bt[:, :nb, :])

                nc.vector.memset(agg[:], 0.0)
                load_gix2(0)
                load_gix2(1)
                state = {}
                for n, (si, s0, ns, so) in enumerate(subs):
                    load_gix2(n + 2)
                    w, j, nwj = slabs[si]
                    if si not in state:
                        nblk = [caps[w][j][gl] // 128 for gl in range(TPW)]
                        pw_new = pp.tile([128, TPW, HID], F32, tag="pw2")
                        state[si] = (
                            pw_new, nblk,
                            [gl for gl in range(TPW)
                             for _ in range(nblk[gl])],
                            [0] * TPW)
                    pw, nblk, gl_of_blk, seen = state[si]
                    gix = gixs.pop(n)
                    mt = mp.tile([128, SUB // 128, 128], BF16, tag="msg2")
                    for q0 in range(0, ns, CALL):
                        nn = min(CALL, ns - q0)
                        nc.gpsimd.dma_gather(
                            mt[:, q0 // 128: (q0 + nn) // 128, :],
                            g2_slices[j], gix[:, q0 // 16: (q0 + nn) // 16],
                            nn, nn, 128, queue_num=next_q())
                    s1t = sp.tile([128, SUB // 128, 128], F8, tag="s12")
                    nc.sync.dma_start(
                        s1t[:, :ns // 128, :], s1d2[:, so: so + ns])
                    for b in range(ns // 128):
                        gl = gl_of_blk[(s0 // 128) + b]
                        nc.tensor.matmul(
                            pw[:, gl, :], s1t[:, b, :], mt[:, b, :HID],
                            start=(seen[gl] == 0),
                            stop=(seen[gl] == nblk[gl] - 1))
                        seen[gl] += 1
                    if s0 + SUB >= nwj:
                        valid = min(TPW, cfg.own_tiles - w * TPW)
                        pwc = mp.tile([128, TPW, HID], F32, tag="pwc2")
                        nc.scalar.copy(pwc[:, :valid], pw[:, :valid, :])
                        nc.vector.tensor_tensor(
                            agg[:, w * TPW: w * TPW + valid, :],
                            agg[:, w * TPW: w * TPW + valid, :],
                            pwc[:, :valid], op=AOp.add)
                        del state[si]
                        if si + 1 == len(slabs) or slabs[si + 1][0] != w:
                            p5_window(w, pw)

            nc.gpsimd.collective_compute(
                "AllGather", mybir.AluOpType.bypass,
                replica_groups=[list(range(NCORES))],
                ins=[As[:]], outs=[Af[:]])

            # ---- P6: edge MLP ----
            # A rows SWDGE-gathered per edge in one merged stream per chunk
            # (calls round-robin the SWDGE queues); B rows expanded to edges
            # by one-hot matmul from the SBUF-resident B table (edges are
            # sorted by (chunk, target tile), so each 128-block has one gt).
            A_slices = [Af[lo:hi, :] for lo, hi in
                        (cfg.chunk_bounds(j) for j in range(cfg.nchunk))]
            SUBM = 4096
            with (
                tc.tile_pool(name="p6i", bufs=4) as ip6,
                tc.tile_pool(name="p6m", bufs=4) as mp6,
                tc.tile_pool(name="p6b", bufs=4) as bp6,
                tc.tile_pool(name="p6bt", bufs=1) as btp,
                tc.tile_pool(name="p6s", bufs=3) as sp6,
                tc.tile_pool(name="p6z", bufs=3) as zp6,
                tc.tile_pool(name="p6p", bufs=2, space="PSUM") as pp6,
            ):
                assert 0 < k_pos < HID, k_pos
                BtAll = btp.tile([128, cfg.own_tiles, HID], BF16, tag="BtA")
                nc.sync.dma_start(
                    BtAll[:], Bl[:].rearrange("(t p) h -> p t h", p=128))
                subs = []
                off = 0
                for j in range(cfg.nchunk):
                    capj = sum(mlp_caps[j])
                    for s0 in range(0, capj, SUBM):
                        subs.append((j, s0, min(SUBM, capj - s0), off + s0))
                    off += capj
                gt_of_blk_j = [
                    [gt for gt in range(cfg.own_tiles)
                     for _ in range(mlp_caps[j][gt] // 128)]
                    for j in range(cfg.nchunk)]
                aixs = {}
                sBs = {}

                def load_aix(n):
                    if n >= len(subs):
                        return
                    _, _, ns, so = subs[n]
                    aix = ip6.tile([128, SUBM // 16], I16, tag="aix")
                    nc.scalar.dma_start(
                        aix[:, :ns // 16],
                        ma[:, so // 16: so // 16 + ns // 16])
                    aixs[n] = aix

                def load_sB(n):
                    if n >= len(subs):
                        return
                    _, _, ns, so = subs[n]
                    sB = bp6.tile([128, SUBM // 128, 128], F8, tag="sB")
                    nc.sync.dma_start(sB[:, :ns // 128, :],
                                      s2d[:, so: so + ns])
                    sBs[n] = sB

                load_aix(0)
                load_aix(1)
                load_sB(0)
                for n, (j, s0, ns, so) in enumerate(subs):
                    load_aix(n + 2)
                    aix = aixs.pop(n)
                    Ag = mp6.tile([128, SUBM // 128, HID], F32, tag="Ag")
                    for q0 in range(0, ns, CALL):
                        nn = min(CALL, ns - q0)
                        nc.gpsimd.dma_gather(
                            Ag[:, q0 // 128: (q0 + nn) // 128, :],
                            A_slices[j], aix[:, q0 // 16: (q0 + nn) // 16],
                            nn, nn, HID, queue_num=next_q())
                    load_sB(n + 1)
                    sB = sBs.pop(n)
                    pB = pp6.tile([128, SUBM // 128, HID], F32, tag="pB")
                    for b in range(ns // 128):
                        gt = gt_of_blk_j[j][s0 // 128 + b]
                        nc.tensor.matmul(pB[:, b, :], sB[:, b, :],
                                         BtAll[:, gt, :],
                                         start=True, stop=True)
                    nb = ns // 128
                    # |w4| is folded into A/B on the host (columns sign-
                    # permuted): score = sum(relu[:k]) - sum(relu[k:]) + b4.
                    # pB leaves PSUM via the scalar engine (vector reads PSUM
                    # at half rate, and a busy vector throttles Q7 desc-gen
                    # through the shared SBUF port).
                    pBs = zp6.tile([128, SUBM // 128, HID], F32, tag="pBs")
                    nc.scalar.copy(pBs[:, :nb], pB[:, :nb])
                    z = Ag[:, :nb, :]
                    nc.vector.tensor_tensor(z, z, pBs[:, :nb], op=AOp.add)
                    zh = zp6.tile([128, SUBM // 128, HID], F32, tag="zh")
                    nc.scalar.activation(
                        zh[:, :nb], z, mybir.ActivationFunctionType.Relu)
                    sc = sp6.tile([128, SUBM // 128], F32, tag="sc")
                    scn = sp6.tile([128, SUBM // 128], F32, tag="scn")
                    nc.vector.tensor_reduce(
                        sc[:, :nb], zh[:, :nb, :k_pos],
                        axis=mybir.AxisListType.X, op=AOp.add)
                    nc.vector.tensor_reduce(
                        scn[:, :nb], zh[:, :nb, k_pos:],
                        axis=mybir.AxisListType.X, op=AOp.add)
                    nc.vector.scalar_tensor_tensor(
                        sc[:, :nb], sc[:, :nb], b4s[:, 0:1], scn[:, :nb],
                        op0=AOp.add, op1=AOp.subtract)
                    nc.sync.dma_start(
                        scores[:, so // 128: so // 128 + nb],
                        sc[:, :nb])
    nc.compile()
    return nc


def host_prep(cfg: Cfg, x, edge_index, W1, b1, W2, b2, W3, b3, W4, b4):
    """Returns (caps, in_maps, out_meta)."""
    N, NPC, CH = cfg.n_nodes, cfg.npc, cfg.chunk
    row = np.asarray(edge_index[0], dtype=np.int64)
    col = np.asarray(edge_index[1], dtype=np.int64)
    E = row.shape[0]
    core = col // NPC
    lc = col - core * NPC

    deg = np.bincount(col, minlength=N).astype(np.int64) + 1

    # rank permutation per core (sort own nodes by local in-degree desc)
    rank_of = np.zeros(N, np.int64)
    for k in range(NCORES):
        ld = np.bincount(lc[core == k], minlength=NPC)
        order = np.argsort(-ld, kind="stable")
        inv = np.empty(NPC, np.int64)
        inv[order] = np.arange(NPC)
        rank_of[k * NPC: (k + 1) * NPC] = inv
    tpos = (np.arange(N) // NPC) * NPC + rank_of
    rowp = tpos[row]
    jch = np.minimum(rowp // CH, cfg.nchunk - 1)
    crank = rank_of[col]          # target id in rank-permuted local layout

    NW = cfg.nwin
    g_of = crank // 128           # in-core node tile 0..own_tiles-1
    w_of = g_of // TPW
    key_all = (w_of * cfg.nchunk + jch) * cfg.own_tiles + g_of

    NKEY = NW * cfg.nchunk * cfg.own_tiles
    # caps shared across cores: max count per (w, j, g), padded to 128
    cnt = np.zeros(NKEY, np.int64)
    for k in range(NCORES):
        cnt = np.maximum(cnt, np.bincount(key_all[core == k], minlength=NKEY))
    capf = -(-cnt // 128) * 128
    caps = [[[0] * TPW for _ in range(cfg.nchunk)] for _ in range(NW)]
    for w in range(NW):
        for j in range(cfg.nchunk):
            for gl in range(TPW):
                g = w * TPW + gl
                if g < cfg.own_tiles:
                    caps[w][j][gl] = int(
                        capf[(w * cfg.nchunk + j) * cfg.own_tiles + g])
    # base slot offset per (w, j, g) in stream order
    # conv1 stream is j-major so it can start on chunk 0 while the g1 table
    # build (P1) is still writing later chunks; conv2 uses a second w-major
    # stream so each PSUM window accumulates across all chunks consecutively
    # and the pointwise h2/A/B work (P5) runs fused per finished window.
    base = np.zeros(NKEY, np.int64)
    acc = 0
    for j in range(cfg.nchunk):
        for w in range(NW):
            for gl in range(TPW):
                g = w * TPW + gl
                if g >= cfg.own_tiles:
                    continue
                kk = (w * cfg.nchunk + j) * cfg.own_tiles + g
                base[kk] = acc
                acc += capf[kk]
    e_conv = acc
    base2 = np.zeros(NKEY, np.int64)
    acc2 = 0
    for w in range(NW):
        for j in range(cfg.nchunk):
            for gl in range(TPW):
                g = w * TPW + gl
                if g >= cfg.own_tiles:
                    continue
                kk = (w * cfg.nchunk + j) * cfg.own_tiles + g
                base2[kk] = acc2
                acc2 += capf[kk]
    assert acc2 == e_conv

    # mlp caps shared across cores: max count per (j, target tile), padded
    key2_all = jch * cfg.own_tiles + g_of
    NK2 = cfg.nchunk * cfg.own_tiles
    cnt2 = np.zeros(NK2, np.int64)
    for k in range(NCORES):
        cnt2 = np.maximum(cnt2, np.bincount(key2_all[core == k], minlength=NK2))
    capf2 = -(-cnt2 // 128) * 128
    mlp_caps = [[int(capf2[j * cfg.own_tiles + g])
                 for g in range(cfg.own_tiles)] for j in range(cfg.nchunk)]
    mlp_base = np.concatenate([[0], np.cumsum(capf2)])[:-1]
    e_mlp = int(capf2.sum())

    def wrap16(vals):
        n = vals.shape[0]
        b = vals.reshape(n // 16, 16).T.astype(np.int16)
        return np.tile(b, (8, 1))

    in_maps = []
    core_of = core
    slot_of = np.zeros(E, np.int64)

    xp = np.zeros((IN_DIM, cfg.tbl_rows), np.float16)
    xp[:, tpos] = np.asarray(x, np.float32).T.astype(np.float16)
    degp = np.ones(cfg.tbl_rows, np.float32)
    degp[tpos] = deg.astype(np.float32)
    degp_w = degp.reshape(cfg.nt, 128).T.copy()

    # fold |w4| into the A/B tables (scale W3 columns + b3) and permute hid
    # channels so w4>=0 channels come first; the device then computes
    # score = sum(relu[:k_pos]) - sum(relu[k_pos:]) + b4 with no multiply.
    w4v = np.asarray(W4, np.float32).reshape(HID)
    perm = np.argsort(w4v < 0, kind="stable")
    k_pos = int((w4v >= 0).sum())
    aw4 = np.abs(w4v[perm])
    consts = {
        "xTp": xp,
        "degp": degp_w,
        "W1": np.asarray(W1, np.float32).astype(np.float16),
        "W2": np.asarray(W2, np.float32),
        "W3t": np.asarray(W3[:HID], np.float32)[:, perm] * aw4[None, :],
        "W3b": np.asarray(W3[HID:], np.float32)[:, perm] * aw4[None, :],
        "b1r": np.tile(np.asarray(b1, np.float32)[None, :], (128, 1)),
        "b2r": np.tile(np.asarray(b2, np.float32)[None, :], (128, 1)),
        "b3r": np.tile((np.asarray(b3, np.float32)[perm] * aw4)[None, :],
                       (128, 1)),
        "w4r": np.tile(np.asarray(W4, np.float32).reshape(1, HID), (128, 1)),
        "b4r": np.full((128, 1), np.float32(np.asarray(b4).reshape(-1)[0])),
        "idm": np.eye(128, dtype=np.float32),
    }

    chunk_lo = np.array([cfg.chunk_bounds(j)[0] for j in range(cfg.nchunk)])
    for k in range(NCORES):
        m = core == k
        eids = np.nonzero(m)[0]
        j_, rk_, rp_, key_ = jch[eids], crank[eids], rowp[eids], key_all[eids]
        key2_ = key2_all[eids]

        # ---- conv streams: slot per edge within its (w, j, g) block ----
        order = np.argsort(key_, kind="stable")
        ks = key_[order]
        uk, inv_, per = np.unique(ks, return_inverse=True, return_counts=True)
        starts = np.concatenate([[0], np.cumsum(per)])[:-1]
        within = np.arange(ks.shape[0]) - starts[inv_]
        lidx = (rp_[order] - chunk_lo[j_[order]]).astype(np.int16)
        tgt = rk_[order] % 128
        slot = base[ks] + within
        cg_v = np.zeros(e_conv, np.int16)
        cg_v[slot] = lidx
        s1_v = np.zeros((128, e_conv), np.float32)
        s1_v[slot % 128, (slot // 128) * 128 + tgt] = 1.0
        slot2c = base2[ks] + within
        cg2_v = np.zeros(e_conv, np.int16)
        cg2_v[slot2c] = lidx
        s12_v = np.zeros((128, e_conv), np.float32)
        s12_v[slot2c % 128, (slot2c // 128) * 128 + tgt] = 1.0

        # ---- mlp stream: order by (chunk, target tile) ----
        order2 = np.argsort(key2_, kind="stable")
        k2s = key2_[order2]
        uk2, inv2, per2 = np.unique(k2s, return_inverse=True,
                                    return_counts=True)
        st2 = np.concatenate([[0], np.cumsum(per2)])[:-1]
        within2 = np.arange(k2s.shape[0]) - st2[inv2]
        slot2 = mlp_base[k2s] + within2
        ma_v = np.zeros(e_mlp, np.int16)
        ma_v[slot2] = (rp_[order2] - chunk_lo[j_[order2]]).astype(np.int16)
        sB_v = np.zeros((128, e_mlp), np.float32)
        sB_v[rk_[order2] % 128, slot2] = 1.0
        slot_of[eids[order2]] = slot2

        # own-core tensors
        own = slice(k * NPC, (k + 1) * NPC)
        xo = np.zeros((IN_DIM, cfg.npc_pad), np.float16)
        xo[:, rank_of[own]] = np.asarray(x, np.float32)[own].T.astype(np.float16)
        dgo = np.ones(cfg.npc_pad, np.float32)
        dgo[rank_of[own]] = deg[own].astype(np.float32)
        dgo_w = dgo.reshape(cfg.own_tiles, 128).T.copy()

        mdict = dict(consts)
        mdict.update({
            "xTo": xo, "dego": dgo_w,
            "cg": wrap16(cg_v),
            "s1d": s1_v.astype(ml_dtypes.float8_e4m3),
            "cg2": wrap16(cg2_v),
            "s1d2": s12_v.astype(ml_dtypes.float8_e4m3),
            "ma": wrap16(ma_v),
            "s2d": sB_v.astype(ml_dtypes.float8_e4m3),
        })
        in_maps.append(mdict)

    return caps, mlp_caps, in_maps, (core_of, slot_of), k_pos


def run(cfg: Cfg, inputs, trace=False):
    from concourse.bass_utils import run_bass_kernel_spmd

    caps, mlp_caps, in_maps, (core_of, slot_of), k_pos = host_prep(
        cfg, **inputs)
    key = "real" if cfg is REAL else id(cfg)
    if key not in _COMPILED:
        _COMPILED[key] = build_program(cfg, caps, mlp_caps, k_pos)
    nc = _COMPILED[key]
    res = run_bass_kernel_spmd(nc, in_maps, list(range(NCORES)),
                               trace=trace)
    sw = np.stack([res.results[k]["scores"] for k in range(NCORES)])
    out = sw[core_of, slot_of % 128, slot_of // 128]
    return out.astype(np.float32), res


def kernel(**inputs) -> np.ndarray:
    out, _ = run(REAL, inputs)
    return out



# revision 60
# speedup vs baseline: 1.0965x; 1.0965x over previous
"""GCN edge classifier on 8 TRN2 NeuronCores (Bass/Tile).

Math (PyG GCNConv x2 + edge MLP), with deg including self-loop:
    dinv = 1/sqrt(deg)
    g1 = dinv * (x @ W1);         agg1[c] = sum_{e: col=c} g1[row_e]
    h1 = relu(dinv * (agg1 + g1) + b1)
    g2 = dinv * (h1 @ W2);        agg2 likewise
    h2 = relu(dinv * (agg2 + g2) + b2)
    score_e = relu(A[row_e] + B[col_e]) @ w4 + b4,
        A = h2 @ W3[:64] + b3,  B = h2 @ W3[64:]

Distribution: edges bucketed by target (col) range -> owning core; each core
owns 12500 nodes.  Conv aggregation: edges sorted by (window, src-chunk,
target-tile); dma_gather (256B bf16 rows, int16 chunk-local indices) pulls
messages into SBUF blocks of 128 edges; a host-built one-hot selection block
[128 edges x 128 targets] is the stationary matmul operand, the message block
the moving operand, so TensorE accumulates agg[128 targets, 64] in PSUM per
node tile (scatter side runs at ~1 cyc/edge instead of ~8 ns/edge SWDGE).
g2/A slices are AllGathered between layers.  Edge MLP unchanged: dma_gather
A/B rows per edge.
"""
from dataclasses import dataclass, field

import numpy as np
import ml_dtypes

NCORES = 8
HID = 64
IN_DIM = 3
TPW = 16          # node tiles per PSUM window
NQ = 4            # SWDGE queues: gather desc-gen round-robins over Q7 core pairs

# MLP caps retained from the fixed-seed inputs (host prep asserts fit).
CAP_MLP_J = [135168, 135168, 135168, 10240]


@dataclass
class Cfg:
    n_nodes: int = 100000
    npc: int = 12500          # nodes per core
    chunk: int = 32768        # gather chunk rows (int16 index range)
    cap_mlp_j: list = field(default_factory=lambda: CAP_MLP_J)
    mlp_blk: int = 16384      # MLP block edges
    p1_grp: int = 8           # table-build tiles per PSUM batch

    @property
    def nchunk(self):
        return -(-self.n_nodes // self.chunk)

    @property
    def npc_pad(self):        # accum ids per core rounded to 128
        return -(-self.npc // 128) * 128

    @property
    def own_tiles(self):
        return self.npc_pad // 128

    @property
    def nwin(self):
        return -(-self.own_tiles // TPW)

    @property
    def e_mlp(self):
        return int(sum(self.cap_mlp_j))

    @property
    def nt(self):             # table tiles (128 rows each), grouped by p1_grp
        raw = -(-self.n_nodes // 128)
        g = self.p1_grp
        return -(-raw // g) * g

    @property
    def tbl_rows(self):
        return self.nt * 128

    def chunk_bounds(self, j):
        lo = j * self.chunk
        return lo, min((j + 1) * self.chunk, self.n_nodes)


REAL = Cfg()

_COMPILED = {}


def build_program(cfg: Cfg, caps, mlp_caps, k_pos):
    """caps[w][j][gl] = padded edge count (multiple of 128) for window w,
    src chunk j, in-window tile gl.  mlp_caps[j][gt] likewise for the edge
    MLP stream keyed by (src chunk, target tile).  k_pos = number of hid
    channels with w4 >= 0 after the host's sign-permutation fold."""
    import concourse.bacc as bacc
    import concourse.mybir as mybir
    import concourse.tile as tile
    from concourse.library_config import mlp as mlp_lib

    F32 = mybir.dt.float32
    BF16 = mybir.dt.float16  # 2-byte msg path: fp16 (8x finer mantissa than bf16)
    F8 = mybir.dt.float8e4   # one-hot selection streams: fp8 is exact for 0/1
    I16 = mybir.dt.int16
    AOp = mybir.AluOpType

    NW = cfg.nwin
    wj_slots = [[sum(caps[w][j]) for j in range(cfg.nchunk)] for w in range(NW)]
    e_conv = int(sum(sum(r) for r in wj_slots))
    maxwj = max(max(r) for r in wj_slots)
    # total matmul blocks per (w, gl) across chunks, for start/stop flags
    blk_tot = [[sum(caps[w][j][gl] // 128 for j in range(cfg.nchunk))
                for gl in range(TPW)] for w in range(NW)]
    e_mlp = int(sum(sum(r) for r in mlp_caps))
    capmax2 = max(max(r) for r in mlp_caps)
    nblk_max = capmax2 // 128

    nc = bacc.Bacc("TRN2", target_bir_lowering=False, debug=False,
                   num_devices=NCORES, num_swdge_queues=NQ)

    # ---- I/O ----
    xTp = nc.dram_tensor("xTp", [IN_DIM, cfg.tbl_rows], BF16, kind="ExternalInput")
    xTo = nc.dram_tensor("xTo", [IN_DIM, cfg.npc_pad], BF16, kind="ExternalInput")
    degp = nc.dram_tensor("degp", [128, cfg.nt], F32, kind="ExternalInput")
    dego = nc.dram_tensor("dego", [128, cfg.own_tiles], F32, kind="ExternalInput")
    W1 = nc.dram_tensor("W1", [IN_DIM, HID], BF16, kind="ExternalInput")
    W2 = nc.dram_tensor("W2", [HID, HID], F32, kind="ExternalInput")
    W3t = nc.dram_tensor("W3t", [HID, HID], F32, kind="ExternalInput")
    W3b = nc.dram_tensor("W3b", [HID, HID], F32, kind="ExternalInput")
    b1r = nc.dram_tensor("b1r", [128, HID], F32, kind="ExternalInput")
    b2r = nc.dram_tensor("b2r", [128, HID], F32, kind="ExternalInput")
    b3r = nc.dram_tensor("b3r", [128, HID], F32, kind="ExternalInput")
    w4r = nc.dram_tensor("w4r", [128, HID], F32, kind="ExternalInput")
    b4r = nc.dram_tensor("b4r", [128, 1], F32, kind="ExternalInput")
    idm = nc.dram_tensor("idm", [128, 128], F32, kind="ExternalInput")
    cg = nc.dram_tensor("cg", [128, e_conv // 16], I16, kind="ExternalInput")
    s1d = nc.dram_tensor("s1d", [128, e_conv], F8, kind="ExternalInput")
    cg2 = nc.dram_tensor("cg2", [128, e_conv // 16], I16, kind="ExternalInput")
    s1d2 = nc.dram_tensor("s1d2", [128, e_conv], F8, kind="ExternalInput")
    ma = nc.dram_tensor("ma", [128, e_mlp // 16], I16, kind="ExternalInput")
    s2d = nc.dram_tensor("s2d", [128, e_mlp], F8, kind="ExternalInput")
    scores = nc.dram_tensor("scores", [128, e_mlp // 128], F32,
                            kind="ExternalOutput")

    # ---- internal DRAM ----
    csz = [cfg.chunk_bounds(j)[1] - cfg.chunk_bounds(j)[0]
           for j in range(cfg.nchunk)]
    ctil = [-(-cfg.tbl_rows // 128) - sum(-(-s // 128) for s in csz[:3])
            if j == cfg.nchunk - 1 else csz[j] // 128
            for j in range(cfg.nchunk)]
    g1c = [nc.dram_tensor(f"g1c{j}", [ctil[j] * 128, 128], BF16)
           for j in range(cfg.nchunk)]
    g2s = nc.dram_tensor("g2s", [cfg.npc, 128], BF16)
    g2f = nc.dram_tensor("g2f", [cfg.n_nodes, 128], BF16, addr_space="Shared")
    As = nc.dram_tensor("As", [cfg.npc, HID], F32)
    Af = nc.dram_tensor("Af", [cfg.n_nodes, HID], F32, addr_space="Shared")
    Bl = nc.dram_tensor("Bl", [cfg.npc_pad, HID], BF16)

    CALL = 1024  # SWDGE descriptor-ring safe per-call index limit

    SUB = 8192  # slots per staged sub-batch

    qrr = [0]  # round-robin SWDGE queue cursor

    def next_q():
        q = qrr[0]
        qrr[0] = (q + 1) % NQ
        return q

    def conv_pass(tc, ip, mp, sp, pp, gsrc_slices, agg):
        # one PSUM accumulation group open at a time: groups are the
        # gl-consecutive blocks within a (w, j) slab; cross-chunk partials
        # are summed into agg (pre-zeroed) on the vector engine.
        # Sub-batches are software-pipelined: gix index loads issue two
        # sub-batches ahead on the scalar HWDGE ring so gathers never wait.
        nc_ = tc.nc
        slabs = []
        off = 0
        for j in range(cfg.nchunk):
            for w in range(NW):
                nwj = wj_slots[w][j]
                if nwj == 0:
                    continue
                slabs.append((w, j, nwj))
                off += nwj
        subs = []
        soff = 0
        for si, (w, j, nwj) in enumerate(slabs):
            for s0 in range(0, nwj, SUB):
                subs.append((si, s0, min(SUB, nwj - s0), soff + s0))
            soff += nwj
        gixs = {}

        def load_gix(n):
            if n >= len(subs):
                return
            _, _, ns, so = subs[n]
            gix = ip.tile([128, SUB // 16], I16, tag="gix")
            nc_.scalar.dma_start(gix[:, :ns // 16],
                                 cg[:, so // 16: so // 16 + ns // 16])
            gixs[n] = gix

        load_gix(0)
        load_gix(1)
        state = {}
        for n, (si, s0, ns, so) in enumerate(subs):
            load_gix(n + 2)
            w, j, nwj = slabs[si]
            if si not in state:
                nblk = [caps[w][j][gl] // 128 for gl in range(TPW)]
                pw_new = pp.tile([128, TPW, HID], F32, tag="pw")
                state[si] = (
                    pw_new, nblk,
                    [gl for gl in range(TPW) for _ in range(nblk[gl])],
                    [0] * TPW)
            pw, nblk, gl_of_blk, seen = state[si]
            gix = gixs.pop(n)
            mt = mp.tile([128, SUB // 128, 128], BF16, tag="msg")
            for q0 in range(0, ns, CALL):
                nn = min(CALL, ns - q0)
                nc_.gpsimd.dma_gather(
                    mt[:, q0 // 128: (q0 + nn) // 128, :],
                    gsrc_slices[j], gix[:, q0 // 16: (q0 + nn) // 16],
                    nn, nn, 128, queue_num=next_q())
            s1t = sp.tile([128, SUB // 128, 128], F8, tag="s1")
            nc_.sync.dma_start(
                s1t[:, :ns // 128, :], s1d[:, so: so + ns])
            for b in range(ns // 128):
                gl = gl_of_blk[(s0 // 128) + b]
                nc_.tensor.matmul(
                    pw[:, gl, :], s1t[:, b, :], mt[:, b, :HID],
                    start=(seen[gl] == 0),
                    stop=(seen[gl] == nblk[gl] - 1))
                seen[gl] += 1
            if s0 + SUB >= nwj:   # slab epilogue
                for gl in range(TPW):
                    t = w * TPW + gl
                    if t >= cfg.own_tiles or nblk[gl] == 0:
                        continue
                    nc_.vector.tensor_tensor(agg[:, t, :], agg[:, t, :],
                                             pw[:, gl, :], op=AOp.add)
                del state[si]

    with tile.TileContext(nc) as tc:
        with (
            tc.tile_pool(name="const", bufs=1) as cp,
            tc.tile_pool(name="acc", bufs=1) as ap_,
            tc.tile_pool(name="dinv", bufs=1) as dp,
        ):
            nc.gpsimd.load_library(mlp_lib)

            W1s = cp.tile([IN_DIM, HID], BF16, tag="W1")
            W2s = cp.tile([HID, HID], F32, tag="W2")
            W3ts = cp.tile([HID, HID], F32, tag="W3t")
            W3bs = cp.tile([HID, HID], F32, tag="W3b")
            b1s = cp.tile([128, HID], F32, tag="b1")
            b2s = cp.tile([128, HID], F32, tag="b2")
            b3s = cp.tile([128, HID], F32, tag="b3")
            w4s = cp.tile([128, HID], F32, tag="w4")
            b4s = cp.tile([128, 1], F32, tag="b4")
            idms = cp.tile([128, 128], F32, tag="idm")
            for t_, d_ in ((W1s, W1), (W2s, W2), (W3ts, W3t), (W3bs, W3b),
                           (b1s, b1r), (b2s, b2r), (b3s, b3r), (w4s, w4r),
                           (b4s, b4r), (idms, idm)):
                nc.sync.dma_start(t_[:], d_[:])

            # dinv tables
            dvp = dp.tile([128, cfg.nt], F32, tag="dvp")
            dvo = dp.tile([128, cfg.own_tiles], F32, tag="dvo")
            for dst, src, n in ((dvp, degp, cfg.nt), (dvo, dego, cfg.own_tiles)):
                tmp = dp.tile([128, n], F32, tag="dtmp")
                nc.sync.dma_start(tmp[:], src[:])
                nc.vector.reciprocal(tmp[:], tmp[:])
                nc.scalar.sqrt(dst[:], tmp[:])

            agg = ap_.tile([128, cfg.own_tiles, HID], F32, tag="agg")
            nc.vector.memset(agg[:], 0.0)

            # ---- P1: build g1 table (replicated, rank-permuted layout) ----
            g1c_t = [t_[:].rearrange("(t p) e -> p t e", p=128)
                     for t_ in g1c]
            PG = cfg.p1_grp
            with (
                tc.tile_pool(name="p1", bufs=3) as p1p,
                tc.tile_pool(name="p1ps", bufs=4, space="PSUM") as p1ps,
            ):
                for tg in range(0, cfg.nt, PG):
                    xt = p1p.tile([IN_DIM, PG * 128], BF16, tag="xt")
                    nc.sync.dma_start(
                        xt[:], xTp[:, tg * 128: (tg + PG) * 128])
                    ps = p1ps.tile([128, PG, HID], F32, tag="p1b")
                    for i in range(PG):
                        nc.tensor.matmul(
                            ps[:, i, :], xt[:, i * 128: (i + 1) * 128], W1s[:])
                    # pad cols 64..127 stay garbage: every consumer reads
                    # only [:, :HID] of gathered rows
                    gt = p1p.tile([128, PG, 128], BF16, tag="g1t")
                    dv_b = dvp[:, tg: tg + PG, None].broadcast_to(
                        [128, PG, HID])
                    nc.vector.tensor_tensor(gt[:, :, :HID], ps[:], dv_b,
                                            op=AOp.mult)
                    jc = (tg * 128) // cfg.chunk
                    tl = tg - jc * (cfg.chunk // 128)
                    nc.sync.dma_start(g1c_t[jc][:, tl: tl + PG, :], gt[:])

            # ---- P2: conv1 aggregation ----
            g1_slices = [g1c[j][:csz[j], :] for j in range(cfg.nchunk)]
            with (
                tc.tile_pool(name="c1i", bufs=4) as ip,
                tc.tile_pool(name="c1m", bufs=4) as mp,
                tc.tile_pool(name="c1s", bufs=4) as sp,
                tc.tile_pool(name="c1p", bufs=2, space="PSUM") as pp,
            ):
                conv_pass(tc, ip, mp, sp, pp, g1_slices, agg)

            # ---- P3: pointwise h1, build g2 slice (batched 8 tiles) ----
            PB = 8
            with (
                tc.tile_pool(name="p3", bufs=3) as p3p,
                tc.tile_pool(name="idmp", bufs=1) as idp,
                tc.tile_pool(name="p3ps", bufs=2, space="PSUM") as p3ps,
            ):
                xto = idp.tile([IN_DIM, cfg.npc_pad], BF16, tag="xto")
                nc.sync.dma_start(xto[:], xTo[:])
                for t0 in range(0, cfg.own_tiles, PB):
                    nb = min(PB, cfg.own_tiles - t0)
                    hw1 = p3ps.tile([128, PB, HID], F32, tag="hw1")
                    for i in range(nb):
                        nc.tensor.matmul(
                            hw1[:, i, :],
                            xto[:, (t0 + i) * 128:(t0 + i + 1) * 128], W1s[:])
                    dv_b = dvo[:, t0:t0 + nb, None].broadcast_to(
                        [128, nb, HID])
                    u = p3p.tile([128, PB, HID], F32, tag="u")
                    nc.vector.tensor_tensor(u[:, :nb], hw1[:, :nb], dv_b,
                                            op=AOp.mult)
                    nc.vector.tensor_tensor(u[:, :nb], u[:, :nb],
                                            agg[:, t0:t0 + nb, :], op=AOp.add)
                    h1 = p3p.tile([128, PB, HID], F32, tag="h1")
                    nc.vector.tensor_tensor(h1[:, :nb], u[:, :nb], dv_b,
                                            op=AOp.mult)
                    b1_b = b1s[:, None, :].broadcast_to([128, nb, HID])
                    nc.vector.tensor_tensor(h1[:, :nb], h1[:, :nb], b1_b,
                                            op=AOp.add)
                    nc.vector.tensor_scalar_max(h1[:, :nb], h1[:, :nb], 0.0)
                    trp = p3ps.tile([HID, PB, 128], F32, tag="trp")
                    for i in range(nb):
                        nc.tensor.transpose(trp[:, i, :], h1[:, i, :], idms[:])
                    h1T = p3p.tile([HID, PB, 128], F32, tag="h1T")
                    nc.scalar.copy(h1T[:, :nb], trp[:, :nb])
                    hw2 = p3ps.tile([128, PB, HID], F32, tag="hw2")
                    for i in range(nb):
                        nc.tensor.matmul(hw2[:, i, :], h1T[:, i, :], W2s[:])
                    g2t = p3p.tile([128, PB, 128], BF16, tag="g2t")
                    nc.vector.tensor_tensor(g2t[:, :nb, :HID], hw2[:, :nb],
                                            dv_b, op=AOp.mult)
                    rows = min(128 * nb, cfg.npc - t0 * 128)
                    fr = rows // 128
                    if fr:
                        nc.sync.dma_start(
                            g2s[t0 * 128: (t0 + fr) * 128, :].rearrange(
                                "(i p) e -> p i e", p=128),
                            g2t[:, :fr, :])
                    rem = rows - fr * 128
                    if rem:
                        nc.sync.dma_start(
                            g2s[(t0 + fr) * 128: t0 * 128 + rows, :],
                            g2t[:rem, fr, :])

            nc.gpsimd.collective_compute(
                "AllGather", mybir.AluOpType.bypass,
                replica_groups=[list(range(NCORES))],
                ins=[g2s[:]], outs=[g2f[:]])

            # ---- P4+P5 fused: conv2 aggregation (w-major stream, window
            # accumulates in PSUM across chunks) + pointwise h2/A/B emitted
            # per finished window, overlapping the next window's gathers ----
            g2_slices = [g2f[lo:hi, :] for lo, hi in
                         (cfg.chunk_bounds(j) for j in range(cfg.nchunk))]
            with (
                tc.tile_pool(name="c2i", bufs=4) as ip,
                tc.tile_pool(name="c2m", bufs=4) as mp,
                tc.tile_pool(name="c2s", bufs=4) as sp,
                tc.tile_pool(name="c2p", bufs=2, space="PSUM") as pp,
                tc.tile_pool(name="p5", bufs=3) as p5p,
                tc.tile_pool(name="p5ps", bufs=1, space="PSUM") as p5ps,
            ):
                slabs = [(w, j, wj_slots[w][j]) for w in range(NW)
                         for j in range(cfg.nchunk) if wj_slots[w][j]]
                subs = []
                soff = 0
                for si, (w, j, nwj) in enumerate(slabs):
                    for s0 in range(0, nwj, SUB):
                        subs.append((si, s0, min(SUB, nwj - s0), soff + s0))
                    soff += nwj
                gixs = {}

                def load_gix2(n):
                    if n >= len(subs):
                        return
                    _, _, ns, so = subs[n]
                    gix = ip.tile([128, SUB // 16], I16, tag="gix2")
                    nc.scalar.dma_start(
                        gix[:, :ns // 16],
                        cg2[:, so // 16: so // 16 + ns // 16])
                    gixs[n] = gix

                def p5_window(w, pw):
                    t_lo = w * TPW
                    t_hi = min(t_lo + TPW, cfg.own_tiles)
                    for t0 in range(t_lo, t_hi, PB):
                        nb = min(PB, t_hi - t0)
                        rows = min(128 * nb, cfg.npc - t0 * 128)
                        fr = rows // 128
                        rem = rows - fr * 128
                        g2o = p5p.tile([128, PB, 128], BF16, tag="g2o")
                        if rem:
                            nc.vector.memset(g2o[:], 0.0)
                        if fr:
                            nc.sync.dma_start(
                                g2o[:, :fr, :],
                                g2s[t0 * 128: (t0 + fr) * 128, :].rearrange(
                                    "(i p) e -> p i e", p=128))
                        if rem:
                            nc.sync.dma_start(
                                g2o[:rem, fr, :],
                                g2s[(t0 + fr) * 128: t0 * 128 + rows, :])
                        g2c = p5p.tile([128, PB, HID], F32, tag="g2c")
                        nc.scalar.copy(g2c[:, :nb], g2o[:, :nb, :HID])
                        u = p5p.tile([128, PB, HID], F32, tag="u5")
                        nc.vector.tensor_tensor(
                            u[:, :nb], g2c[:, :nb],
                            agg[:, t0:t0 + nb, :], op=AOp.add)
                        dv_b = dvo[:, t0:t0 + nb, None].broadcast_to(
                            [128, nb, HID])
                        h2 = p5p.tile([128, PB, HID], F32, tag="h2")
                        nc.vector.tensor_tensor(h2[:, :nb], u[:, :nb], dv_b,
                                                op=AOp.mult)
                        b2_b = b2s[:, None, :].broadcast_to([128, nb, HID])
                        nc.vector.tensor_tensor(h2[:, :nb], h2[:, :nb], b2_b,
                                                op=AOp.add)
                        nc.vector.tensor_scalar_max(h2[:, :nb], h2[:, :nb],
                                                    0.0)
                        trp = p5ps.tile([HID, PB, 128], F32, tag="trp5")
                        for i in range(nb):
                            nc.tensor.transpose(trp[:, i, :], h2[:, i, :],
                                                idms[:])
                        h2T = p5p.tile([HID, PB, 128], F32, tag="h2T")
                        nc.scalar.copy(h2T[:, :nb], trp[:, :nb])
                        psA = p5ps.tile([128, PB, HID], F32, tag="psA")
                        for i in range(nb):
                            nc.tensor.matmul(psA[:, i, :], h2T[:, i, :],
                                             W3ts[:])
                        At = p5p.tile([128, PB, HID], F32, tag="At")
                        b3_b = b3s[:, None, :].broadcast_to([128, nb, HID])
                        nc.vector.tensor_tensor(At[:, :nb], psA[:, :nb], b3_b,
                                                op=AOp.add)
                        if fr:
                            nc.sync.dma_start(
                                As[t0 * 128: (t0 + fr) * 128, :].rearrange(
                                    "(i p) h -> p i h", p=128),
                                At[:, :fr, :])
                        if rem:
                            nc.sync.dma_start(
                                As[(t0 + fr) * 128: t0 * 128 + rows, :],
                                At[:rem, fr, :])
                        psB = p5ps.tile([128, PB, HID], F32, tag="psB")
                        for i in range(nb):
                            nc.tensor.matmul(psB[:, i, :], h2T[:, i, :],
                                             W3bs[:])
                        Bt = p5p.tile([128, PB, HID], BF16, tag="Bt")
                        nc.vector.tensor_copy(Bt[:, :nb], psB[:, :nb])
                        nc.sync.dma_start(
                            Bl[t0 * 128: (t0 + nb) * 128, :].rearrange(
                                "(i p) h -> p i h", p=128),
                            Bt[:, :nb, :])

                nc.vector.memset(agg[:], 0.0)
                load_gix2(0)
                load_gix2(1)
                state = {}
                for n, (si, s0, ns, so) in enumerate(subs):
                    load_gix2(n + 2)
                    w, j, nwj = slabs[si]
                    if si not in state:
                        nblk = [caps[w][j][gl] // 128 for gl in range(TPW)]
                        pw_new = pp.tile([128, TPW, HID], F32, tag="pw2")
                        state[si] = (
                            pw_new, nblk,
                            [gl for gl in range(TPW)
                             for _ in range(nblk[gl])],
                            [0] * TPW)
                    pw, nblk, gl_of_blk, seen = state[si]
                    gix = gixs.pop(n)
                    mt = mp.tile([128, SUB // 128, 128], BF16, tag="msg2")
                    for q0 in range(0, ns, CALL):
                        nn = min(CALL, ns - q0)
                        nc.gpsimd.dma_gather(
                            mt[:, q0 // 128: (q0 + nn) // 128, :],
                            g2_slices[j], gix[:, q0 // 16: (q0 + nn) // 16],
                            nn, nn, 128, queue_num=next_q())
                    s1t = sp.tile([128, SUB // 128, 128], F8, tag="s12")
                    nc.sync.dma_start(
                        s1t[:, :ns // 128, :], s1d2[:, so: so + ns])
                    for b in range(ns // 128):
                        gl = gl_of_blk[(s0 // 128) + b]
                        nc.tensor.matmul(
                            pw[:, gl, :], s1t[:, b, :], mt[:, b, :HID],
                            start=(seen[gl] == 0),
                            stop=(seen[gl] == nblk[gl] - 1))
                        seen[gl] += 1
                    if s0 + SUB >= nwj:
                        for gl in range(TPW):
                            t = w * TPW + gl
                            if t >= cfg.own_tiles or nblk[gl] == 0:
                                continue
                            nc.vector.tensor_tensor(
                                agg[:, t, :], agg[:, t, :], pw[:, gl, :],
                                op=AOp.add)
                        del state[si]
                        if si + 1 == len(slabs) or slabs[si + 1][0] != w:
                            p5_window(w, pw)

            nc.gpsimd.collective_compute(
                "AllGather", mybir.AluOpType.bypass,
                replica_groups=[list(range(NCORES))],
                ins=[As[:]], outs=[Af[:]])

            # ---- P6: edge MLP ----
            # A rows SWDGE-gathered per edge in one merged stream per chunk
            # (calls round-robin the SWDGE queues); B rows expanded to edges
            # by one-hot matmul from the SBUF-resident B table (edges are
            # sorted by (chunk, target tile), so each 128-block has one gt).
            A_slices = [Af[lo:hi, :] for lo, hi in
                        (cfg.chunk_bounds(j) for j in range(cfg.nchunk))]
            SUBM = 4096
            with (
                tc.tile_pool(name="p6i", bufs=4) as ip6,
                tc.tile_pool(name="p6m", bufs=4) as mp6,
                tc.tile_pool(name="p6b", bufs=4) as bp6,
                tc.tile_pool(name="p6bt", bufs=1) as btp,
                tc.tile_pool(name="p6s", bufs=3) as sp6,
                tc.tile_pool(name="p6z", bufs=3) as zp6,
                tc.tile_pool(name="p6p", bufs=2, space="PSUM") as pp6,
            ):
                assert 0 < k_pos < HID, k_pos
                BtAll = btp.tile([128, cfg.own_tiles, HID], BF16, tag="BtA")
                nc.sync.dma_start(
                    BtAll[:], Bl[:].rearrange("(t p) h -> p t h", p=128))
                subs = []
                off = 0
                for j in range(cfg.nchunk):
                    capj = sum(mlp_caps[j])
                    for s0 in range(0, capj, SUBM):
                        subs.append((j, s0, min(SUBM, capj - s0), off + s0))
                    off += capj
                gt_of_blk_j = [
                    [gt for gt in range(cfg.own_tiles)
                     for _ in range(mlp_caps[j][gt] // 128)]
                    for j in range(cfg.nchunk)]
                aixs = {}
                sBs = {}

                def load_aix(n):
                    if n >= len(subs):
                        return
                    _, _, ns, so = subs[n]
                    aix = ip6.tile([128, SUBM // 16], I16, tag="aix")
                    nc.scalar.dma_start(
                        aix[:, :ns // 16],
                        ma[:, so // 16: so // 16 + ns // 16])
                    aixs[n] = aix

                def load_sB(n):
                    if n >= len(subs):
                        return
                    _, _, ns, so = subs[n]
                    sB = bp6.tile([128, SUBM // 128, 128], F8, tag="sB")
                    nc.sync.dma_start(sB[:, :ns // 128, :],
                                      s2d[:, so: so + ns])
                    sBs[n] = sB

                load_aix(0)
                load_aix(1)
                load_sB(0)
                for n, (j, s0, ns, so) in enumerate(subs):
                    load_aix(n + 2)
                    aix = aixs.pop(n)
                    Ag = mp6.tile([128, SUBM // 128, HID], F32, tag="Ag")
                    for q0 in range(0, ns, CALL):
                        nn = min(CALL, ns - q0)
                        nc.gpsimd.dma_gather(
                            Ag[:, q0 // 128: (q0 + nn) // 128, :],
                            A_slices[j], aix[:, q0 // 16: (q0 + nn) // 16],
                            nn, nn, HID, queue_num=next_q())
                    load_sB(n + 1)
                    sB = sBs.pop(n)
                    pB = pp6.tile([128, SUBM // 128, HID], F32, tag="pB")
                    for b in range(ns // 128):
                        gt = gt_of_blk_j[j][s0 // 128 + b]
                        nc.tensor.matmul(pB[:, b, :], sB[:, b, :],
                                         BtAll[:, gt, :],
                                         start=True, stop=True)
                    nb = ns // 128
                    # |w4| is folded into A/B on the host (columns sign-
                    # permuted): score = sum(relu[:k]) - sum(relu[k:]) + b4.
                    # pB leaves PSUM via the scalar engine (vector reads PSUM
                    # at half rate, and a busy vector throttles Q7 desc-gen
                    # through the shared SBUF port).
                    # add lands in pBs (not in-place on Ag) so Ag's last
                    # reader is the vector add — gathers reusing the Ag ring
                    # then never wait on the scalar relu chain
                    pBs = zp6.tile([128, SUBM // 128, HID], F32, tag="pBs")
                    nc.scalar.copy(pBs[:, :nb], pB[:, :nb])
                    nc.vector.tensor_tensor(pBs[:, :nb], pBs[:, :nb],
                                            Ag[:, :nb, :], op=AOp.add)
                    zh = zp6.tile([128, SUBM // 128, HID], F32, tag="zh")
                    nc.scalar.activation(
                        zh[:, :nb], pBs[:, :nb],
                        mybir.ActivationFunctionType.Relu)
                    sc = sp6.tile([128, SUBM // 128], F32, tag="sc")
                    scn = sp6.tile([128, SUBM // 128], F32, tag="scn")
                    nc.vector.tensor_reduce(
                        sc[:, :nb], zh[:, :nb, :k_pos],
                        axis=mybir.AxisListType.X, op=AOp.add)
                    nc.vector.tensor_reduce(
                        scn[:, :nb], zh[:, :nb, k_pos:],
                        axis=mybir.AxisListType.X, op=AOp.add)
                    nc.vector.scalar_tensor_tensor(
                        sc[:, :nb], sc[:, :nb], b4s[:, 0:1], scn[:, :nb],
                        op0=AOp.add, op1=AOp.subtract)
                    nc.sync.dma_start(
                        scores[:, so // 128: so // 128 + nb],
                        sc[:, :nb])
    nc.compile()
    return nc


def host_prep(cfg: Cfg, x, edge_index, W1, b1, W2, b2, W3, b3, W4, b4):
    """Returns (caps, in_maps, out_meta)."""
    N, NPC, CH = cfg.n_nodes, cfg.npc, cfg.chunk
    row = np.asarray(edge_index[0], dtype=np.int64)
    col = np.asarray(edge_index[1], dtype=np.int64)
    E = row.shape[0]
    core = col // NPC
    lc = col - core * NPC

    deg = np.bincount(col, minlength=N).astype(np.int64) + 1

    # rank permutation per core (sort own nodes by local in-degree desc)
    rank_of = np.zeros(N, np.int64)
    for k in range(NCORES):
        ld = np.bincount(lc[core == k], minlength=NPC)
        order = np.argsort(-ld, kind="stable")
        inv = np.empty(NPC, np.int64)
        inv[order] = np.arange(NPC)
        rank_of[k * NPC: (k + 1) * NPC] = inv
    tpos = (np.arange(N) // NPC) * NPC + rank_of
    rowp = tpos[row]
    jch = np.minimum(rowp // CH, cfg.nchunk - 1)
    crank = rank_of[col]          # target id in rank-permuted local layout

    NW = cfg.nwin
    g_of = crank // 128           # in-core node tile 0..own_tiles-1
    w_of = g_of // TPW
    key_all = (w_of * cfg.nchunk + jch) * cfg.own_tiles + g_of

    NKEY = NW * cfg.nchunk * cfg.own_tiles
    # caps shared across cores: max count per (w, j, g), padded to 128
    cnt = np.zeros(NKEY, np.int64)
    for k in range(NCORES):
        cnt = np.maximum(cnt, np.bincount(key_all[core == k], minlength=NKEY))
    capf = -(-cnt // 128) * 128
    caps = [[[0] * TPW for _ in range(cfg.nchunk)] for _ in range(NW)]
    for w in range(NW):
        for j in range(cfg.nchunk):
            for gl in range(TPW):
                g = w * TPW + gl
                if g < cfg.own_tiles:
                    caps[w][j][gl] = int(
                        capf[(w * cfg.nchunk + j) * cfg.own_tiles + g])
    # base slot offset per (w, j, g) in stream order
    # conv1 stream is j-major so it can start on chunk 0 while the g1 table
    # build (P1) is still writing later chunks; conv2 uses a second w-major
    # stream so each PSUM window accumulates across all chunks consecutively
    # and the pointwise h2/A/B work (P5) runs fused per finished window.
    base = np.zeros(NKEY, np.int64)
    acc = 0
    for j in range(cfg.nchunk):
        for w in range(NW):
            for gl in range(TPW):
                g = w * TPW + gl
                if g >= cfg.own_tiles:
                    continue
                kk = (w * cfg.nchunk + j) * cfg.own_tiles + g
                base[kk] = acc
                acc += capf[kk]
    e_conv = acc
    base2 = np.zeros(NKEY, np.int64)
    acc2 = 0
    for w in range(NW):
        for j in range(cfg.nchunk):
            for gl in range(TPW):
                g = w * TPW + gl
                if g >= cfg.own_tiles:
                    continue
                kk = (w * cfg.nchunk + j) * cfg.own_tiles + g
                base2[kk] = acc2
                acc2 += capf[kk]
    assert acc2 == e_conv

    # mlp caps shared across cores: max count per (j, target tile), padded
    key2_all = jch * cfg.own_tiles + g_of
    NK2 = cfg.nchunk * cfg.own_tiles
    cnt2 = np.zeros(NK2, np.int64)
    for k in range(NCORES):
        cnt2 = np.maximum(cnt2, np.bincount(key2_all[core == k], minlength=NK2))
    capf2 = -(-cnt2 // 128) * 128
    mlp_caps = [[int(capf2[j * cfg.own_tiles + g])
                 for g in range(cfg.own_tiles)] for j in range(cfg.nchunk)]
    mlp_base = np.concatenate([[0], np.cumsum(capf2)])[:-1]
    e_mlp = int(capf2.sum())

    def wrap16(vals):
        n = vals.shape[0]
        b = vals.reshape(n // 16, 16).T.astype(np.int16)
        return np.tile(b, (8, 1))

    in_maps = []
    core_of = core
    slot_of = np.zeros(E, np.int64)

    xp = np.zeros((IN_DIM, cfg.tbl_rows), np.float16)
    xp[:, tpos] = np.asarray(x, np.float32).T.astype(np.float16)
    degp = np.ones(cfg.tbl_rows, np.float32)
    degp[tpos] = deg.astype(np.float32)
    degp_w = degp.reshape(cfg.nt, 128).T.copy()

    # fold |w4| into the A/B tables (scale W3 columns + b3) and permute hid
    # channels so w4>=0 channels come first; the device then computes
    # score = sum(relu[:k_pos]) - sum(relu[k_pos:]) + b4 with no multiply.
    w4v = np.asarray(W4, np.float32).reshape(HID)
    perm = np.argsort(w4v < 0, kind="stable")
    k_pos = int((w4v >= 0).sum())
    aw4 = np.abs(w4v[perm])
    consts = {
        "xTp": xp,
        "degp": degp_w,
        "W1": np.asarray(W1, np.float32).astype(np.float16),
        "W2": np.asarray(W2, np.float32),
        "W3t": np.asarray(W3[:HID], np.float32)[:, perm] * aw4[None, :],
        "W3b": np.asarray(W3[HID:], np.float32)[:, perm] * aw4[None, :],
        "b1r": np.tile(np.asarray(b1, np.float32)[None, :], (128, 1)),
        "b2r": np.tile(np.asarray(b2, np.float32)[None, :], (128, 1)),
        "b3r": np.tile((np.asarray(b3, np.float32)[perm] * aw4)[None, :],
                       (128, 1)),
        "w4r": np.tile(np.asarray(W4, np.float32).reshape(1, HID), (128, 1)),
        "b4r": np.full((128, 1), np.float32(np.asarray(b4).reshape(-1)[0])),
        "idm": np.eye(128, dtype=np.float32),
    }

    chunk_lo = np.array([cfg.chunk_bounds(j)[0] for j in range(cfg.nchunk)])
    for k in range(NCORES):
        m = core == k
        eids = np.nonzero(m)[0]
        j_, rk_, rp_, key_ = jch[eids], crank[eids], rowp[eids], key_all[eids]
        key2_ = key2_all[eids]

        # ---- conv streams: slot per edge within its (w, j, g) block ----
        order = np.argsort(key_, kind="stable")
        ks = key_[order]
        uk, inv_, per = np.unique(ks, return_inverse=True, return_counts=True)
        starts = np.concatenate([[0], np.cumsum(per)])[:-1]
        within = np.arange(ks.shape[0]) - starts[inv_]
        lidx = (rp_[order] - chunk_lo[j_[order]]).astype(np.int16)
        tgt = rk_[order] % 128
        slot = base[ks] + within
        cg_v = np.zeros(e_conv, np.int16)
        cg_v[slot] = lidx
        s1_v = np.zeros((128, e_conv), np.float32)
        s1_v[slot % 128, (slot // 128) * 128 + tgt] = 1.0
        slot2c = base2[ks] + within
        cg2_v = np.zeros(e_conv, np.int16)
        cg2_v[slot2c] = lidx
        s12_v = np.zeros((128, e_conv), np.float32)
        s12_v[slot2c % 128, (slot2c // 128) * 128 + tgt] = 1.0

        # ---- mlp stream: order by (chunk, target tile) ----
        order2 = np.argsort(key2_, kind="stable")
        k2s = key2_[order2]
        uk2, inv2, per2 = np.unique(k2s, return_inverse=True,
                                    return_counts=True)
        st2 = np.concatenate([[0], np.cumsum(per2)])[:-1]
        within2 = np.arange(k2s.shape[0]) - st2[inv2]
        slot2 = mlp_base[k2s] + within2
        ma_v = np.zeros(e_mlp, np.int16)
        ma_v[slot2] = (rp_[order2] - chunk_lo[j_[order2]]).astype(np.int16)
        sB_v = np.zeros((128, e_mlp), np.float32)
        sB_v[rk_[order2] % 128, slot2] = 1.0
        slot_of[eids[order2]] = slot2

        # own-core tensors
        own = slice(k * NPC, (k + 1) * NPC)
        xo = np.zeros((IN_DIM, cfg.npc_pad), np.float16)
        xo[:, rank_of[own]] = np.asarray(x, np.float32)[own].T.astype(np.float16)
        dgo = np.ones(cfg.npc_pad, np.float32)
        dgo[rank_of[own]] = deg[own].astype(np.float32)
        dgo_w = dgo.reshape(cfg.own_tiles, 128).T.copy()

        mdict = dict(consts)
        mdict.update({
            "xTo": xo, "dego": dgo_w,
            "cg": wrap16(cg_v),
            "s1d": s1_v.astype(ml_dtypes.float8_e4m3),
            "cg2": wrap16(cg2_v),
            "s1d2": s12_v.astype(ml_dtypes.float8_e4m3),
            "ma": wrap16(ma_v),
            "s2d": sB_v.astype(ml_dtypes.float8_e4m3),
        })
        in_maps.append(mdict)

    return caps, mlp_caps, in_maps, (core_of, slot_of), k_pos


def run(cfg: Cfg, inputs, trace=False):
    from concourse.bass_utils import run_bass_kernel_spmd

    caps, mlp_caps, in_maps, (core_of, slot_of), k_pos = host_prep(
        cfg, **inputs)
    key = "real" if cfg is REAL else id(cfg)
    if key not in _COMPILED:
        _COMPILED[key] = build_program(cfg, caps, mlp_caps, k_pos)
    nc = _COMPILED[key]
    res = run_bass_kernel_spmd(nc, in_maps, list(range(NCORES)),
                               trace=trace)
    sw = np.stack([res.results[k]["scores"] for k in range(NCORES)])
    out = sw[core_of, slot_of % 128, slot_of // 128]
    return out.astype(np.float32), res


def kernel(**inputs) -> np.ndarray:
    out, _ = run(REAL, inputs)
    return out



# revision 61
# speedup vs baseline: 1.1004x; 1.0035x over previous
"""GCN edge classifier on 8 TRN2 NeuronCores (Bass/Tile).

Math (PyG GCNConv x2 + edge MLP), with deg including self-loop:
    dinv = 1/sqrt(deg)
    g1 = dinv * (x @ W1);         agg1[c] = sum_{e: col=c} g1[row_e]
    h1 = relu(dinv * (agg1 + g1) + b1)
    g2 = dinv * (h1 @ W2);        agg2 likewise
    h2 = relu(dinv * (agg2 + g2) + b2)
    score_e = relu(A[row_e] + B[col_e]) @ w4 + b4,
        A = h2 @ W3[:64] + b3,  B = h2 @ W3[64:]

Distribution: edges bucketed by target (col) range -> owning core; each core
owns 12500 nodes.  Conv aggregation: edges sorted by (window, src-chunk,
target-tile); dma_gather (256B bf16 rows, int16 chunk-local indices) pulls
messages into SBUF blocks of 128 edges; a host-built one-hot selection block
[128 edges x 128 targets] is the stationary matmul operand, the message block
the moving operand, so TensorE accumulates agg[128 targets, 64] in PSUM per
node tile (scatter side runs at ~1 cyc/edge instead of ~8 ns/edge SWDGE).
g2/A slices are AllGathered between layers.  Edge MLP unchanged: dma_gather
A/B rows per edge.
"""
from dataclasses import dataclass, field

import numpy as np
import ml_dtypes

NCORES = 8
HID = 64
IN_DIM = 3
TPW = 16          # node tiles per PSUM window
NQ = 4            # SWDGE queues: gather desc-gen round-robins over Q7 core pairs

# MLP caps retained from the fixed-seed inputs (host prep asserts fit).
CAP_MLP_J = [135168, 135168, 135168, 10240]


@dataclass
class Cfg:
    n_nodes: int = 100000
    npc: int = 12500          # nodes per core
    chunk: int = 32768        # gather chunk rows (int16 index range)
    cap_mlp_j: list = field(default_factory=lambda: CAP_MLP_J)
    mlp_blk: int = 16384      # MLP block edges
    p1_grp: int = 8           # table-build tiles per PSUM batch

    @property
    def nchunk(self):
        return -(-self.n_nodes // self.chunk)

    @property
    def npc_pad(self):        # accum ids per core rounded to 128
        return -(-self.npc // 128) * 128

    @property
    def own_tiles(self):
        return self.npc_pad // 128

    @property
    def nwin(self):
        return -(-self.own_tiles // TPW)

    @property
    def e_mlp(self):
        return int(sum(self.cap_mlp_j))

    @property
    def nt(self):             # table tiles (128 rows each), grouped by p1_grp
        raw = -(-self.n_nodes // 128)
        g = self.p1_grp
        return -(-raw // g) * g

    @property
    def tbl_rows(self):
        return self.nt * 128

    def chunk_bounds(self, j):
        lo = j * self.chunk
        return lo, min((j + 1) * self.chunk, self.n_nodes)


REAL = Cfg()

_COMPILED = {}


def build_program(cfg: Cfg, caps, mlp_caps, k_pos):
    """caps[w][j][gl] = padded edge count (multiple of 128) for window w,
    src chunk j, in-window tile gl.  mlp_caps[j][gt] likewise for the edge
    MLP stream keyed by (src chunk, target tile).  k_pos = number of hid
    channels with w4 >= 0 after the host's sign-permutation fold."""
    import concourse.bacc as bacc
    import concourse.mybir as mybir
    import concourse.tile as tile
    from concourse.library_config import mlp as mlp_lib

    F32 = mybir.dt.float32
    BF16 = mybir.dt.float16  # 2-byte msg path: fp16 (8x finer mantissa than bf16)
    F8 = mybir.dt.float8e4   # one-hot selection streams: fp8 is exact for 0/1
    I16 = mybir.dt.int16
    AOp = mybir.AluOpType

    NW = cfg.nwin
    wj_slots = [[sum(caps[w][j]) for j in range(cfg.nchunk)] for w in range(NW)]
    e_conv = int(sum(sum(r) for r in wj_slots))
    maxwj = max(max(r) for r in wj_slots)
    # total matmul blocks per (w, gl) across chunks, for start/stop flags
    blk_tot = [[sum(caps[w][j][gl] // 128 for j in range(cfg.nchunk))
                for gl in range(TPW)] for w in range(NW)]
    e_mlp = int(sum(sum(r) for r in mlp_caps))
    capmax2 = max(max(r) for r in mlp_caps)
    nblk_max = capmax2 // 128

    nc = bacc.Bacc("TRN2", target_bir_lowering=False, debug=False,
                   num_devices=NCORES, num_swdge_queues=NQ)

    # ---- I/O ----
    xTp = nc.dram_tensor("xTp", [IN_DIM, cfg.tbl_rows], BF16, kind="ExternalInput")
    xTo = nc.dram_tensor("xTo", [IN_DIM, cfg.npc_pad], BF16, kind="ExternalInput")
    degp = nc.dram_tensor("degp", [128, cfg.nt], F32, kind="ExternalInput")
    dego = nc.dram_tensor("dego", [128, cfg.own_tiles], F32, kind="ExternalInput")
    W1 = nc.dram_tensor("W1", [IN_DIM, HID], BF16, kind="ExternalInput")
    W2 = nc.dram_tensor("W2", [HID, HID], F32, kind="ExternalInput")
    W3t = nc.dram_tensor("W3t", [HID, HID], F32, kind="ExternalInput")
    W3b = nc.dram_tensor("W3b", [HID, HID], F32, kind="ExternalInput")
    b1r = nc.dram_tensor("b1r", [128, HID], F32, kind="ExternalInput")
    b2r = nc.dram_tensor("b2r", [128, HID], F32, kind="ExternalInput")
    b3r = nc.dram_tensor("b3r", [128, HID], F32, kind="ExternalInput")
    w4r = nc.dram_tensor("w4r", [128, HID], F32, kind="ExternalInput")
    b4r = nc.dram_tensor("b4r", [128, 1], F32, kind="ExternalInput")
    idm = nc.dram_tensor("idm", [128, 128], F32, kind="ExternalInput")
    cg = nc.dram_tensor("cg", [128, e_conv // 16], I16, kind="ExternalInput")
    s1d = nc.dram_tensor("s1d", [128, e_conv], F8, kind="ExternalInput")
    cg2 = nc.dram_tensor("cg2", [128, e_conv // 16], I16, kind="ExternalInput")
    s1d2 = nc.dram_tensor("s1d2", [128, e_conv], F8, kind="ExternalInput")
    ma = nc.dram_tensor("ma", [128, e_mlp // 16], I16, kind="ExternalInput")
    s2d = nc.dram_tensor("s2d", [128, e_mlp], F8, kind="ExternalInput")
    scores = nc.dram_tensor("scores", [128, e_mlp // 128], F32,
                            kind="ExternalOutput")

    # ---- internal DRAM ----
    csz = [cfg.chunk_bounds(j)[1] - cfg.chunk_bounds(j)[0]
           for j in range(cfg.nchunk)]
    ctil = [-(-cfg.tbl_rows // 128) - sum(-(-s // 128) for s in csz[:3])
            if j == cfg.nchunk - 1 else csz[j] // 128
            for j in range(cfg.nchunk)]
    g1c = [nc.dram_tensor(f"g1c{j}", [ctil[j] * 128, 128], BF16)
           for j in range(cfg.nchunk)]
    g2s = nc.dram_tensor("g2s", [cfg.npc, 128], BF16)
    g2f = nc.dram_tensor("g2f", [cfg.n_nodes, 128], BF16, addr_space="Shared")
    As = nc.dram_tensor("As", [cfg.npc, HID], F32)
    Af = nc.dram_tensor("Af", [cfg.n_nodes, HID], F32, addr_space="Shared")
    Bl = nc.dram_tensor("Bl", [cfg.npc_pad, HID], BF16)

    CALL = 1024  # SWDGE descriptor-ring safe per-call index limit

    SUB = 8192  # slots per staged sub-batch

    qrr = [0]  # round-robin SWDGE queue cursor

    def next_q():
        q = qrr[0]
        qrr[0] = (q + 1) % NQ
        return q

    def conv_pass(tc, ip, mp, sp, pp, gsrc_slices, agg):
        # one PSUM accumulation group open at a time: groups are the
        # gl-consecutive blocks within a (w, j) slab; cross-chunk partials
        # are summed into agg (pre-zeroed) on the vector engine.
        # Sub-batches are software-pipelined: gix index loads issue two
        # sub-batches ahead on the scalar HWDGE ring so gathers never wait.
        nc_ = tc.nc
        slabs = []
        off = 0
        for j in range(cfg.nchunk):
            for w in range(NW):
                nwj = wj_slots[w][j]
                if nwj == 0:
                    continue
                slabs.append((w, j, nwj))
                off += nwj
        subs = []
        soff = 0
        for si, (w, j, nwj) in enumerate(slabs):
            for s0 in range(0, nwj, SUB):
                subs.append((si, s0, min(SUB, nwj - s0), soff + s0))
            soff += nwj
        gixs = {}

        def load_gix(n):
            if n >= len(subs):
                return
            _, _, ns, so = subs[n]
            gix = ip.tile([128, SUB // 16], I16, tag="gix")
            nc_.scalar.dma_start(gix[:, :ns // 16],
                                 cg[:, so // 16: so // 16 + ns // 16])
            gixs[n] = gix

        load_gix(0)
        load_gix(1)
        state = {}
        for n, (si, s0, ns, so) in enumerate(subs):
            load_gix(n + 2)
            w, j, nwj = slabs[si]
            if si not in state:
                nblk = [caps[w][j][gl] // 128 for gl in range(TPW)]
                pw_new = pp.tile([128, TPW, HID], F32, tag="pw")
                state[si] = (
                    pw_new, nblk,
                    [gl for gl in range(TPW) for _ in range(nblk[gl])],
                    [0] * TPW)
            pw, nblk, gl_of_blk, seen = state[si]
            gix = gixs.pop(n)
            mt = mp.tile([128, SUB // 128, 128], BF16, tag="msg")
            for q0 in range(0, ns, CALL):
                nn = min(CALL, ns - q0)
                nc_.gpsimd.dma_gather(
                    mt[:, q0 // 128: (q0 + nn) // 128, :],
                    gsrc_slices[j], gix[:, q0 // 16: (q0 + nn) // 16],
                    nn, nn, 128, queue_num=next_q())
            s1t = sp.tile([128, SUB // 128, 128], F8, tag="s1")
            nc_.sync.dma_start(
                s1t[:, :ns // 128, :], s1d[:, so: so + ns])
            for b in range(ns // 128):
                gl = gl_of_blk[(s0 // 128) + b]
                nc_.tensor.matmul(
                    pw[:, gl, :], s1t[:, b, :], mt[:, b, :HID],
                    start=(seen[gl] == 0),
                    stop=(seen[gl] == nblk[gl] - 1))
                seen[gl] += 1
            if s0 + SUB >= nwj:   # slab epilogue
                for gl in range(TPW):
                    t = w * TPW + gl
                    if t >= cfg.own_tiles or nblk[gl] == 0:
                        continue
                    nc_.vector.tensor_tensor(agg[:, t, :], agg[:, t, :],
                                             pw[:, gl, :], op=AOp.add)
                del state[si]

    with tile.TileContext(nc) as tc:
        with (
            tc.tile_pool(name="const", bufs=1) as cp,
            tc.tile_pool(name="acc", bufs=1) as ap_,
            tc.tile_pool(name="dinv", bufs=1) as dp,
        ):
            nc.gpsimd.load_library(mlp_lib)

            W1s = cp.tile([IN_DIM, HID], BF16, tag="W1")
            W2s = cp.tile([HID, HID], F32, tag="W2")
            W3ts = cp.tile([HID, HID], F32, tag="W3t")
            W3bs = cp.tile([HID, HID], F32, tag="W3b")
            b1s = cp.tile([128, HID], F32, tag="b1")
            b2s = cp.tile([128, HID], F32, tag="b2")
            b3s = cp.tile([128, HID], F32, tag="b3")
            w4s = cp.tile([128, HID], F32, tag="w4")
            b4s = cp.tile([128, 1], F32, tag="b4")
            idms = cp.tile([128, 128], F32, tag="idm")
            for t_, d_ in ((W1s, W1), (W2s, W2), (W3ts, W3t), (W3bs, W3b),
                           (b1s, b1r), (b2s, b2r), (b3s, b3r), (w4s, w4r),
                           (b4s, b4r), (idms, idm)):
                nc.sync.dma_start(t_[:], d_[:])

            # dinv tables
            dvp = dp.tile([128, cfg.nt], F32, tag="dvp")
            dvo = dp.tile([128, cfg.own_tiles], F32, tag="dvo")
            for dst, src, n in ((dvp, degp, cfg.nt), (dvo, dego, cfg.own_tiles)):
                tmp = dp.tile([128, n], F32, tag="dtmp")
                nc.sync.dma_start(tmp[:], src[:])
                nc.vector.reciprocal(tmp[:], tmp[:])
                nc.scalar.sqrt(dst[:], tmp[:])

            agg = ap_.tile([128, cfg.own_tiles, HID], F32, tag="agg")
            nc.vector.memset(agg[:], 0.0)

            # ---- P1: build g1 table (replicated, rank-permuted layout) ----
            g1c_t = [t_[:].rearrange("(t p) e -> p t e", p=128)
                     for t_ in g1c]
            PG = cfg.p1_grp
            with (
                tc.tile_pool(name="p1", bufs=3) as p1p,
                tc.tile_pool(name="p1ps", bufs=4, space="PSUM") as p1ps,
            ):
                for tg in range(0, cfg.nt, PG):
                    xt = p1p.tile([IN_DIM, PG * 128], BF16, tag="xt")
                    nc.sync.dma_start(
                        xt[:], xTp[:, tg * 128: (tg + PG) * 128])
                    ps = p1ps.tile([128, PG, HID], F32, tag="p1b")
                    for i in range(PG):
                        nc.tensor.matmul(
                            ps[:, i, :], xt[:, i * 128: (i + 1) * 128], W1s[:])
                    # pad cols 64..127 stay garbage: every consumer reads
                    # only [:, :HID] of gathered rows
                    gt = p1p.tile([128, PG, 128], BF16, tag="g1t")
                    dv_b = dvp[:, tg: tg + PG, None].broadcast_to(
                        [128, PG, HID])
                    nc.vector.tensor_tensor(gt[:, :, :HID], ps[:], dv_b,
                                            op=AOp.mult)
                    jc = (tg * 128) // cfg.chunk
                    tl = tg - jc * (cfg.chunk // 128)
                    nc.sync.dma_start(g1c_t[jc][:, tl: tl + PG, :], gt[:])

            # ---- P2: conv1 aggregation ----
            g1_slices = [g1c[j][:csz[j], :] for j in range(cfg.nchunk)]
            with (
                tc.tile_pool(name="c1i", bufs=4) as ip,
                tc.tile_pool(name="c1m", bufs=4) as mp,
                tc.tile_pool(name="c1s", bufs=4) as sp,
                tc.tile_pool(name="c1p", bufs=2, space="PSUM") as pp,
            ):
                conv_pass(tc, ip, mp, sp, pp, g1_slices, agg)

            # ---- P3: pointwise h1, build g2 slice (batched 8 tiles) ----
            PB = 8
            with (
                tc.tile_pool(name="p3", bufs=3) as p3p,
                tc.tile_pool(name="idmp", bufs=1) as idp,
                tc.tile_pool(name="p3ps", bufs=2, space="PSUM") as p3ps,
            ):
                xto = idp.tile([IN_DIM, cfg.npc_pad], BF16, tag="xto")
                nc.sync.dma_start(xto[:], xTo[:])
                for t0 in range(0, cfg.own_tiles, PB):
                    nb = min(PB, cfg.own_tiles - t0)
                    hw1 = p3ps.tile([128, PB, HID], F32, tag="hw1")
                    for i in range(nb):
                        nc.tensor.matmul(
                            hw1[:, i, :],
                            xto[:, (t0 + i) * 128:(t0 + i + 1) * 128], W1s[:])
                    dv_b = dvo[:, t0:t0 + nb, None].broadcast_to(
                        [128, nb, HID])
                    u = p3p.tile([128, PB, HID], F32, tag="u")
                    nc.vector.tensor_tensor(u[:, :nb], hw1[:, :nb], dv_b,
                                            op=AOp.mult)
                    nc.vector.tensor_tensor(u[:, :nb], u[:, :nb],
                                            agg[:, t0:t0 + nb, :], op=AOp.add)
                    h1 = p3p.tile([128, PB, HID], F32, tag="h1")
                    nc.vector.tensor_tensor(h1[:, :nb], u[:, :nb], dv_b,
                                            op=AOp.mult)
                    b1_b = b1s[:, None, :].broadcast_to([128, nb, HID])
                    nc.vector.tensor_tensor(h1[:, :nb], h1[:, :nb], b1_b,
                                            op=AOp.add)
                    nc.vector.tensor_scalar_max(h1[:, :nb], h1[:, :nb], 0.0)
                    trp = p3ps.tile([HID, PB, 128], F32, tag="trp")
                    for i in range(nb):
                        nc.tensor.transpose(trp[:, i, :], h1[:, i, :], idms[:])
                    h1T = p3p.tile([HID, PB, 128], F32, tag="h1T")
                    nc.scalar.copy(h1T[:, :nb], trp[:, :nb])
                    hw2 = p3ps.tile([128, PB, HID], F32, tag="hw2")
                    for i in range(nb):
                        nc.tensor.matmul(hw2[:, i, :], h1T[:, i, :], W2s[:])
                    g2t = p3p.tile([128, PB, 128], BF16, tag="g2t")
                    nc.vector.tensor_tensor(g2t[:, :nb, :HID], hw2[:, :nb],
                                            dv_b, op=AOp.mult)
                    rows = min(128 * nb, cfg.npc - t0 * 128)
                    fr = rows // 128
                    if fr:
                        nc.sync.dma_start(
                            g2s[t0 * 128: (t0 + fr) * 128, :].rearrange(
                                "(i p) e -> p i e", p=128),
                            g2t[:, :fr, :])
                    rem = rows - fr * 128
                    if rem:
                        nc.sync.dma_start(
                            g2s[(t0 + fr) * 128: t0 * 128 + rows, :],
                            g2t[:rem, fr, :])

            nc.gpsimd.collective_compute(
                "AllGather", mybir.AluOpType.bypass,
                replica_groups=[list(range(NCORES))],
                ins=[g2s[:]], outs=[g2f[:]])

            # ---- P4+P5 fused: conv2 aggregation (w-major stream, window
            # accumulates in PSUM across chunks) + pointwise h2/A/B emitted
            # per finished window, overlapping the next window's gathers ----
            g2_slices = [g2f[lo:hi, :] for lo, hi in
                         (cfg.chunk_bounds(j) for j in range(cfg.nchunk))]
            with (
                tc.tile_pool(name="c2i", bufs=4) as ip,
                tc.tile_pool(name="c2m", bufs=4) as mp,
                tc.tile_pool(name="c2s", bufs=4) as sp,
                tc.tile_pool(name="c2p", bufs=2, space="PSUM") as pp,
                tc.tile_pool(name="p5", bufs=3) as p5p,
                tc.tile_pool(name="p5ps", bufs=1, space="PSUM") as p5ps,
            ):
                slabs = [(w, j, wj_slots[w][j]) for w in range(NW)
                         for j in range(cfg.nchunk) if wj_slots[w][j]]
                subs = []
                soff = 0
                for si, (w, j, nwj) in enumerate(slabs):
                    for s0 in range(0, nwj, SUB):
                        subs.append((si, s0, min(SUB, nwj - s0), soff + s0))
                    soff += nwj
                gixs = {}

                def load_gix2(n):
                    if n >= len(subs):
                        return
                    _, _, ns, so = subs[n]
                    gix = ip.tile([128, SUB // 16], I16, tag="gix2")
                    nc.scalar.dma_start(
                        gix[:, :ns // 16],
                        cg2[:, so // 16: so // 16 + ns // 16])
                    gixs[n] = gix

                def p5_window(w, pw):
                    t_lo = w * TPW
                    t_hi = min(t_lo + TPW, cfg.own_tiles)
                    for t0 in range(t_lo, t_hi, PB):
                        nb = min(PB, t_hi - t0)
                        rows = min(128 * nb, cfg.npc - t0 * 128)
                        fr = rows // 128
                        rem = rows - fr * 128
                        g2o = p5p.tile([128, PB, 128], BF16, tag="g2o")
                        if rem:
                            nc.vector.memset(g2o[:], 0.0)
                        if fr:
                            nc.sync.dma_start(
                                g2o[:, :fr, :],
                                g2s[t0 * 128: (t0 + fr) * 128, :].rearrange(
                                    "(i p) e -> p i e", p=128))
                        if rem:
                            nc.sync.dma_start(
                                g2o[:rem, fr, :],
                                g2s[(t0 + fr) * 128: t0 * 128 + rows, :])
                        g2c = p5p.tile([128, PB, HID], F32, tag="g2c")
                        nc.scalar.copy(g2c[:, :nb], g2o[:, :nb, :HID])
                        u = p5p.tile([128, PB, HID], F32, tag="u5")
                        nc.vector.tensor_tensor(
                            u[:, :nb], g2c[:, :nb],
                            agg[:, t0:t0 + nb, :], op=AOp.add)
                        dv_b = dvo[:, t0:t0 + nb, None].broadcast_to(
                            [128, nb, HID])
                        h2 = p5p.tile([128, PB, HID], F32, tag="h2")
                        nc.vector.tensor_tensor(h2[:, :nb], u[:, :nb], dv_b,
                                                op=AOp.mult)
                        b2_b = b2s[:, None, :].broadcast_to([128, nb, HID])
                        nc.vector.tensor_tensor(h2[:, :nb], h2[:, :nb], b2_b,
                                                op=AOp.add)
                        nc.vector.tensor_scalar_max(h2[:, :nb], h2[:, :nb],
                                                    0.0)
                        trp = p5ps.tile([HID, PB, 128], F32, tag="trp5")
                        for i in range(nb):
                            nc.tensor.transpose(trp[:, i, :], h2[:, i, :],
                                                idms[:])
                        h2T = p5p.tile([HID, PB, 128], F32, tag="h2T")
                        nc.scalar.copy(h2T[:, :nb], trp[:, :nb])
                        psA = p5ps.tile([128, PB, HID], F32, tag="psA")
                        for i in range(nb):
                            nc.tensor.matmul(psA[:, i, :], h2T[:, i, :],
                                             W3ts[:])
                        At = p5p.tile([128, PB, HID], F32, tag="At")
                        b3_b = b3s[:, None, :].broadcast_to([128, nb, HID])
                        nc.vector.tensor_tensor(At[:, :nb], psA[:, :nb], b3_b,
                                                op=AOp.add)
                        if fr:
                            nc.sync.dma_start(
                                As[t0 * 128: (t0 + fr) * 128, :].rearrange(
                                    "(i p) h -> p i h", p=128),
                                At[:, :fr, :])
                        if rem:
                            nc.sync.dma_start(
                                As[(t0 + fr) * 128: t0 * 128 + rows, :],
                                At[:rem, fr, :])
                        psB = p5ps.tile([128, PB, HID], F32, tag="psB")
                        for i in range(nb):
                            nc.tensor.matmul(psB[:, i, :], h2T[:, i, :],
                                             W3bs[:])
                        Bt = p5p.tile([128, PB, HID], BF16, tag="Bt")
                        nc.vector.tensor_copy(Bt[:, :nb], psB[:, :nb])
                        nc.sync.dma_start(
                            Bl[t0 * 128: (t0 + nb) * 128, :].rearrange(
                                "(i p) h -> p i h", p=128),
                            Bt[:, :nb, :])

                nc.vector.memset(agg[:], 0.0)
                load_gix2(0)
                load_gix2(1)
                state = {}
                for n, (si, s0, ns, so) in enumerate(subs):
                    load_gix2(n + 2)
                    w, j, nwj = slabs[si]
                    if si not in state:
                        nblk = [caps[w][j][gl] // 128 for gl in range(TPW)]
                        pw_new = pp.tile([128, TPW, HID], F32, tag="pw2")
                        state[si] = (
                            pw_new, nblk,
                            [gl for gl in range(TPW)
                             for _ in range(nblk[gl])],
                            [0] * TPW)
                    pw, nblk, gl_of_blk, seen = state[si]
                    gix = gixs.pop(n)
                    mt = mp.tile([128, SUB // 128, 128], BF16, tag="msg2")
                    for q0 in range(0, ns, CALL):
                        nn = min(CALL, ns - q0)
                        nc.gpsimd.dma_gather(
                            mt[:, q0 // 128: (q0 + nn) // 128, :],
                            g2_slices[j], gix[:, q0 // 16: (q0 + nn) // 16],
                            nn, nn, 128, queue_num=next_q())
                    s1t = sp.tile([128, SUB // 128, 128], F8, tag="s12")
                    nc.sync.dma_start(
                        s1t[:, :ns // 128, :], s1d2[:, so: so + ns])
                    for b in range(ns // 128):
                        gl = gl_of_blk[(s0 // 128) + b]
                        nc.tensor.matmul(
                            pw[:, gl, :], s1t[:, b, :], mt[:, b, :HID],
                            start=(seen[gl] == 0),
                            stop=(seen[gl] == nblk[gl] - 1))
                        seen[gl] += 1
                    if s0 + SUB >= nwj:
                        for gl in range(TPW):
                            t = w * TPW + gl
                            if t >= cfg.own_tiles or nblk[gl] == 0:
                                continue
                            nc.vector.tensor_tensor(
                                agg[:, t, :], agg[:, t, :], pw[:, gl, :],
                                op=AOp.add)
                        del state[si]
                        if si + 1 == len(slabs) or slabs[si + 1][0] != w:
                            p5_window(w, pw)

            nc.gpsimd.collective_compute(
                "AllGather", mybir.AluOpType.bypass,
                replica_groups=[list(range(NCORES))],
                ins=[As[:]], outs=[Af[:]])

            # ---- P6: edge MLP ----
            # A rows SWDGE-gathered per edge in one merged stream per chunk
            # (calls round-robin the SWDGE queues); B rows expanded to edges
            # by one-hot matmul from the SBUF-resident B table (edges are
            # sorted by (chunk, target tile), so each 128-block has one gt).
            A_slices = [Af[lo:hi, :] for lo, hi in
                        (cfg.chunk_bounds(j) for j in range(cfg.nchunk))]
            SUBM = 4096
            with (
                tc.tile_pool(name="p6i", bufs=4) as ip6,
                tc.tile_pool(name="p6m", bufs=6) as mp6,
                tc.tile_pool(name="p6b", bufs=4) as bp6,
                tc.tile_pool(name="p6bt", bufs=1) as btp,
                tc.tile_pool(name="p6s", bufs=3) as sp6,
                tc.tile_pool(name="p6z", bufs=4) as zp6,
                tc.tile_pool(name="p6p", bufs=2, space="PSUM") as pp6,
            ):
                assert 0 < k_pos < HID, k_pos
                BtAll = btp.tile([128, cfg.own_tiles, HID], BF16, tag="BtA")
                nc.sync.dma_start(
                    BtAll[:], Bl[:].rearrange("(t p) h -> p t h", p=128))
                subs = []
                off = 0
                for j in range(cfg.nchunk):
                    capj = sum(mlp_caps[j])
                    for s0 in range(0, capj, SUBM):
                        subs.append((j, s0, min(SUBM, capj - s0), off + s0))
                    off += capj
                gt_of_blk_j = [
                    [gt for gt in range(cfg.own_tiles)
                     for _ in range(mlp_caps[j][gt] // 128)]
                    for j in range(cfg.nchunk)]
                aixs = {}
                sBs = {}

                def load_aix(n):
                    if n >= len(subs):
                        return
                    _, _, ns, so = subs[n]
                    aix = ip6.tile([128, SUBM // 16], I16, tag="aix")
                    nc.scalar.dma_start(
                        aix[:, :ns // 16],
                        ma[:, so // 16: so // 16 + ns // 16])
                    aixs[n] = aix

                def load_sB(n):
                    if n >= len(subs):
                        return
                    _, _, ns, so = subs[n]
                    sB = bp6.tile([128, SUBM // 128, 128], F8, tag="sB")
                    nc.sync.dma_start(sB[:, :ns // 128, :],
                                      s2d[:, so: so + ns])
                    sBs[n] = sB

                load_aix(0)
                load_aix(1)
                load_sB(0)
                for n, (j, s0, ns, so) in enumerate(subs):
                    load_aix(n + 2)
                    aix = aixs.pop(n)
                    Ag = mp6.tile([128, SUBM // 128, HID], F32, tag="Ag")
                    for q0 in range(0, ns, CALL):
                        nn = min(CALL, ns - q0)
                        nc.gpsimd.dma_gather(
                            Ag[:, q0 // 128: (q0 + nn) // 128, :],
                            A_slices[j], aix[:, q0 // 16: (q0 + nn) // 16],
                            nn, nn, HID, queue_num=next_q())
                    load_sB(n + 1)
                    sB = sBs.pop(n)
                    pB = pp6.tile([128, SUBM // 128, HID], F32, tag="pB")
                    for b in range(ns // 128):
                        gt = gt_of_blk_j[j][s0 // 128 + b]
                        nc.tensor.matmul(pB[:, b, :], sB[:, b, :],
                                         BtAll[:, gt, :],
                                         start=True, stop=True)
                    nb = ns // 128
                    # |w4| is folded into A/B on the host (columns sign-
                    # permuted): score = sum(relu[:k]) - sum(relu[k:]) + b4.
                    # pB leaves PSUM via the scalar engine (vector reads PSUM
                    # at half rate, and a busy vector throttles Q7 desc-gen
                    # through the shared SBUF port).
                    # add lands in pBs (not in-place on Ag) so Ag's last
                    # reader is the vector add — gathers reusing the Ag ring
                    # then never wait on the scalar relu chain
                    pBs = zp6.tile([128, SUBM // 128, HID], F32, tag="pBs")
                    nc.scalar.copy(pBs[:, :nb], pB[:, :nb])
                    nc.vector.tensor_tensor(pBs[:, :nb], pBs[:, :nb],
                                            Ag[:, :nb, :], op=AOp.add)
                    zh = zp6.tile([128, SUBM // 128, HID], F32, tag="zh")
                    nc.scalar.activation(
                        zh[:, :nb], pBs[:, :nb],
                        mybir.ActivationFunctionType.Relu)
                    sc = sp6.tile([128, SUBM // 128], F32, tag="sc")
                    scn = sp6.tile([128, SUBM // 128], F32, tag="scn")
                    nc.vector.tensor_reduce(
                        sc[:, :nb], zh[:, :nb, :k_pos],
                        axis=mybir.AxisListType.X, op=AOp.add)
                    nc.vector.tensor_reduce(
                        scn[:, :nb], zh[:, :nb, k_pos:],
                        axis=mybir.AxisListType.X, op=AOp.add)
                    nc.vector.scalar_tensor_tensor(
                        sc[:, :nb], sc[:, :nb], b4s[:, 0:1], scn[:, :nb],
                        op0=AOp.add, op1=AOp.subtract)
                    nc.sync.dma_start(
                        scores[:, so // 128: so // 128 + nb],
                        sc[:, :nb])
    nc.compile()
    return nc


def host_prep(cfg: Cfg, x, edge_index, W1, b1, W2, b2, W3, b3, W4, b4):
    """Returns (caps, in_maps, out_meta)."""
    N, NPC, CH = cfg.n_nodes, cfg.npc, cfg.chunk
    row = np.asarray(edge_index[0], dtype=np.int64)
    col = np.asarray(edge_index[1], dtype=np.int64)
    E = row.shape[0]
    core = col // NPC
    lc = col - core * NPC

    deg = np.bincount(col, minlength=N).astype(np.int64) + 1

    # rank permutation per core (sort own nodes by local in-degree desc)
    rank_of = np.zeros(N, np.int64)
    for k in range(NCORES):
        ld = np.bincount(lc[core == k], minlength=NPC)
        order = np.argsort(-ld, kind="stable")
        inv = np.empty(NPC, np.int64)
        inv[order] = np.arange(NPC)
        rank_of[k * NPC: (k + 1) * NPC] = inv
    tpos = (np.arange(N) // NPC) * NPC + rank_of
    rowp = tpos[row]
    jch = np.minimum(rowp // CH, cfg.nchunk - 1)
    crank = rank_of[col]          # target id in rank-permuted local layout

    NW = cfg.nwin
    g_of = crank // 128           # in-core node tile 0..own_tiles-1
    w_of = g_of // TPW
    key_all = (w_of * cfg.nchunk + jch) * cfg.own_tiles + g_of

    NKEY = NW * cfg.nchunk * cfg.own_tiles
    # caps shared across cores: max count per (w, j, g), padded to 128
    cnt = np.zeros(NKEY, np.int64)
    for k in range(NCORES):
        cnt = np.maximum(cnt, np.bincount(key_all[core == k], minlength=NKEY))
    capf = -(-cnt // 128) * 128
    caps = [[[0] * TPW for _ in range(cfg.nchunk)] for _ in range(NW)]
    for w in range(NW):
        for j in range(cfg.nchunk):
            for gl in range(TPW):
                g = w * TPW + gl
                if g < cfg.own_tiles:
                    caps[w][j][gl] = int(
                        capf[(w * cfg.nchunk + j) * cfg.own_tiles + g])
    # base slot offset per (w, j, g) in stream order
    # conv1 stream is j-major so it can start on chunk 0 while the g1 table
    # build (P1) is still writing later chunks; conv2 uses a second w-major
    # stream so each PSUM window accumulates across all chunks consecutively
    # and the pointwise h2/A/B work (P5) runs fused per finished window.
    base = np.zeros(NKEY, np.int64)
    acc = 0
    for j in range(cfg.nchunk):
        for w in range(NW):
            for gl in range(TPW):
                g = w * TPW + gl
                if g >= cfg.own_tiles:
                    continue
                kk = (w * cfg.nchunk + j) * cfg.own_tiles + g
                base[kk] = acc
                acc += capf[kk]
    e_conv = acc
    base2 = np.zeros(NKEY, np.int64)
    acc2 = 0
    for w in range(NW):
        for j in range(cfg.nchunk):
            for gl in range(TPW):
                g = w * TPW + gl
                if g >= cfg.own_tiles:
                    continue
                kk = (w * cfg.nchunk + j) * cfg.own_tiles + g
                base2[kk] = acc2
                acc2 += capf[kk]
    assert acc2 == e_conv

    # mlp caps shared across cores: max count per (j, target tile), padded
    key2_all = jch * cfg.own_tiles + g_of
    NK2 = cfg.nchunk * cfg.own_tiles
    cnt2 = np.zeros(NK2, np.int64)
    for k in range(NCORES):
        cnt2 = np.maximum(cnt2, np.bincount(key2_all[core == k], minlength=NK2))
    capf2 = -(-cnt2 // 128) * 128
    mlp_caps = [[int(capf2[j * cfg.own_tiles + g])
                 for g in range(cfg.own_tiles)] for j in range(cfg.nchunk)]
    mlp_base = np.concatenate([[0], np.cumsum(capf2)])[:-1]
    e_mlp = int(capf2.sum())

    def wrap16(vals):
        n = vals.shape[0]
        b = vals.reshape(n // 16, 16).T.astype(np.int16)
        return np.tile(b, (8, 1))

    in_maps = []
    core_of = core
    slot_of = np.zeros(E, np.int64)

    xp = np.zeros((IN_DIM, cfg.tbl_rows), np.float16)
    xp[:, tpos] = np.asarray(x, np.float32).T.astype(np.float16)
    degp = np.ones(cfg.tbl_rows, np.float32)
    degp[tpos] = deg.astype(np.float32)
    degp_w = degp.reshape(cfg.nt, 128).T.copy()

    # fold |w4| into the A/B tables (scale W3 columns + b3) and permute hid
    # channels so w4>=0 channels come first; the device then computes
    # score = sum(relu[:k_pos]) - sum(relu[k_pos:]) + b4 with no multiply.
    w4v = np.asarray(W4, np.float32).reshape(HID)
    perm = np.argsort(w4v < 0, kind="stable")
    k_pos = int((w4v >= 0).sum())
    aw4 = np.abs(w4v[perm])
    consts = {
        "xTp": xp,
        "degp": degp_w,
        "W1": np.asarray(W1, np.float32).astype(np.float16),
        "W2": np.asarray(W2, np.float32),
        "W3t": np.asarray(W3[:HID], np.float32)[:, perm] * aw4[None, :],
        "W3b": np.asarray(W3[HID:], np.float32)[:, perm] * aw4[None, :],
        "b1r": np.tile(np.asarray(b1, np.float32)[None, :], (128, 1)),
        "b2r": np.tile(np.asarray(b2, np.float32)[None, :], (128, 1)),
        "b3r": np.tile((np.asarray(b3, np.float32)[perm] * aw4)[None, :],
                       (128, 1)),
        "w4r": np.tile(np.asarray(W4, np.float32).reshape(1, HID), (128, 1)),
        "b4r": np.full((128, 1), np.float32(np.asarray(b4).reshape(-1)[0])),
        "idm": np.eye(128, dtype=np.float32),
    }

    chunk_lo = np.array([cfg.chunk_bounds(j)[0] for j in range(cfg.nchunk)])
    for k in range(NCORES):
        m = core == k
        eids = np.nonzero(m)[0]
        j_, rk_, rp_, key_ = jch[eids], crank[eids], rowp[eids], key_all[eids]
        key2_ = key2_all[eids]

        # ---- conv streams: slot per edge within its (w, j, g) block ----
        order = np.argsort(key_, kind="stable")
        ks = key_[order]
        uk, inv_, per = np.unique(ks, return_inverse=True, return_counts=True)
        starts = np.concatenate([[0], np.cumsum(per)])[:-1]
        within = np.arange(ks.shape[0]) - starts[inv_]
        lidx = (rp_[order] - chunk_lo[j_[order]]).astype(np.int16)
        tgt = rk_[order] % 128
        slot = base[ks] + within
        cg_v = np.zeros(e_conv, np.int16)
        cg_v[slot] = lidx
        s1_v = np.zeros((128, e_conv), np.float32)
        s1_v[slot % 128, (slot // 128) * 128 + tgt] = 1.0
        slot2c = base2[ks] + within
        cg2_v = np.zeros(e_conv, np.int16)
        cg2_v[slot2c] = lidx
        s12_v = np.zeros((128, e_conv), np.float32)
        s12_v[slot2c % 128, (slot2c // 128) * 128 + tgt] = 1.0

        # ---- mlp stream: order by (chunk, target tile) ----
        order2 = np.argsort(key2_, kind="stable")
        k2s = key2_[order2]
        uk2, inv2, per2 = np.unique(k2s, return_inverse=True,
                                    return_counts=True)
        st2 = np.concatenate([[0], np.cumsum(per2)])[:-1]
        within2 = np.arange(k2s.shape[0]) - st2[inv2]
        slot2 = mlp_base[k2s] + within2
        ma_v = np.zeros(e_mlp, np.int16)
        ma_v[slot2] = (rp_[order2] - chunk_lo[j_[order2]]).astype(np.int16)
        sB_v = np.zeros((128, e_mlp), np.float32)
        sB_v[rk_[order2] % 128, slot2] = 1.0
        slot_of[eids[order2]] = slot2

        # own-core tensors
        own = slice(k * NPC, (k + 1) * NPC)
        xo = np.zeros((IN_DIM, cfg.npc_pad), np.float16)
        xo[:, rank_of[own]] = np.asarray(x, np.float32)[own].T.astype(np.float16)
        dgo = np.ones(cfg.npc_pad, np.float32)
        dgo[rank_of[own]] = deg[own].astype(np.float32)
        dgo_w = dgo.reshape(cfg.own_tiles, 128).T.copy()

        mdict = dict(consts)
        mdict.update({
            "xTo": xo, "dego": dgo_w,
            "cg": wrap16(cg_v),
            "s1d": s1_v.astype(ml_dtypes.float8_e4m3),
            "cg2": wrap16(cg2_v),
            "s1d2": s12_v.astype(ml_dtypes.float8_e4m3),
            "ma": wrap16(ma_v),
            "s2d": sB_v.astype(ml_dtypes.float8_e4m3),
        })
        in_maps.append(mdict)

    return caps, mlp_caps, in_maps, (core_of, slot_of), k_pos


def run(cfg: Cfg, inputs, trace=False):
    from concourse.bass_utils import run_bass_kernel_spmd

    caps, mlp_caps, in_maps, (core_of, slot_of), k_pos = host_prep(
        cfg, **inputs)
    key = "real" if cfg is REAL else id(cfg)
    if key not in _COMPILED:
        _COMPILED[key] = build_program(cfg, caps, mlp_caps, k_pos)
    nc = _COMPILED[key]
    res = run_bass_kernel_spmd(nc, in_maps, list(range(NCORES)),
                               trace=trace)
    sw = np.stack([res.results[k]["scores"] for k in range(NCORES)])
    out = sw[core_of, slot_of % 128, slot_of // 128]
    return out.astype(np.float32), res


def kernel(**inputs) -> np.ndarray:
    out, _ = run(REAL, inputs)
    return out

